# revision 3
# baseline (speedup 1.0000x reference)
"""Trainium2 Bass kernel for nn_AdjunctionModel (segment_reduce).

Math (per point, N=1e6 points, B=64 sorted segments):
    h1   = relu(pos @ Wf1 + bf1)            (N,128)
    aff  = h1 @ Wf2 + bf2                   (N,16)   [output]
    h2   = relu(aff @ Wg1 + bg1)            (N,128)
    rec  = h2 @ Wg2 + bg2                   (N,3)    [output]
    err  = sum((pos - rec)^2, -1)           (N,)     [output]
    per-segment means of err and aff feed a tiny GRU (B=64).

Key algebraic fold: there is no nonlinearity between the two middle
matmuls, so  h2 = relu(h1 @ (Wf2 @ Wg1) + (bf2 @ Wg1 + bg1)).  The
device computes, per 512-point block:
    L1   : h1 = Wf1^T @ posT            (fp32r matmul, N=512)
    relu1: s1 = relu(h1 + bf1)          (ACT, fp16 out)
    L3   : pre2 = W23^T @ s1            (fp16 matmul)
    relu2: s2 = relu(pre2 + b23)        (ACT or DVE, alternating)
    aff0 : Wf2p^T @ s1  -> quadrant 32q of a psum tile   (fp16)
    rec0 : Wg2p^T @ s2  -> quadrant 32q of a second tile (fp16)
Per triad (3 blocks), a DVE 32x32 StreamTranspose turns the quadrant-
packed (96,512) psum tiles into per-point-layout and the per-window
(2 triads) partial sums are reduced by a ones-matmul.  Host combines
per-window partials into per-segment sums (recomputing the few windows
that straddle a segment boundary from the per-point outputs), adds the
deferred biases, and runs the tiny GRU in numpy.

Sharding: data-parallel over points, 8 cores, same NEFF on every core
(per-core inputs differ only in data).
"""

import os
import sys
from contextlib import ExitStack

import numpy as np

sys.path.insert(0, "/opt/trn_rl_repo")

import ml_dtypes  # noqa: E402
import concourse.bass as bass  # noqa: E402
import concourse.tile as tile  # noqa: E402
from concourse import bacc, mybir  # noqa: E402
from concourse.bass_utils import run_bass_kernel_spmd  # noqa: E402

F32 = mybir.dt.float32
F32R = mybir.dt.float32r
F16 = mybir.dt.float16
AF = mybir.ActivationFunctionType
ALU = mybir.AluOpType
AX = mybir.AxisListType

N = 1_000_000
B = 64
NCORES = 8
NC = N // NCORES           # 125000 points per core
BLOCK = 512
TRIAD = 3 * BLOCK          # 1536
WINDOW = 2 * TRIAD         # 3072
NW = (NC + WINDOW - 1) // WINDOW   # 41
NPAD = NW * WINDOW         # 125952
NTRIAD = 2 * NW            # 82

# relu2 engine assignment: ACT on even blocks, DVE on odd (≈0.5 split)
def _relu2_on_act(blk: int) -> bool:
    return blk % 2 == 0


_CACHE = {}


def _build_program():
    if "prog" in _CACHE:
        return _CACHE["prog"]

    nc = bacc.Bacc("TRN2", target_bir_lowering=False, debug=False,
                   num_devices=NCORES)

    # ---- DRAM I/O ----
    d_posT = nc.dram_tensor("posT", [3, NPAD], F32, kind="ExternalInput").ap()
    d_posb = nc.dram_tensor("posb", [NTRIAD, 96, 48], F32,
                            kind="ExternalInput").ap()
    d_Wf1 = nc.dram_tensor("Wf1", [3, 128], F32, kind="ExternalInput").ap()
    d_W23 = nc.dram_tensor("W23", [128, 128], F16, kind="ExternalInput").ap()
    d_Wf2p = nc.dram_tensor("Wf2p", [128, 32], F16, kind="ExternalInput").ap()
    d_Wg2p = nc.dram_tensor("Wg2p", [128, 32], F16, kind="ExternalInput").ap()
    d_bf1 = nc.dram_tensor("bf1", [128, 1], F32, kind="ExternalInput").ap()
    d_b23 = nc.dram_tensor("b23", [128, 1], F32, kind="ExternalInput").ap()

    d_aff = nc.dram_tensor("aff", [NTRIAD, 96, 16, 16], F32,
                           kind="ExternalOutput").ap()
    d_diff = nc.dram_tensor("diff", [NTRIAD, 96, 48], F32,
                            kind="ExternalOutput").ap()
    d_errp = nc.dram_tensor("errp", [NW, 96, 32], F32,
                            kind="ExternalOutput").ap()
    d_part = nc.dram_tensor("part", [1, NW * 288], F32,
                            kind="ExternalOutput").ap()

    with tile.TileContext(nc) as tc, ExitStack() as ctx:
        consts = ctx.enter_context(tc.tile_pool(name="consts", bufs=1))
        pposT = ctx.enter_context(tc.tile_pool(name="pposT", bufs=2))
        pposb = ctx.enter_context(tc.tile_pool(name="pposb", bufs=2))
        ps1 = ctx.enter_context(tc.tile_pool(name="ps1", bufs=2))
        ps2 = ctx.enter_context(tc.tile_pool(name="ps2", bufs=2))
        pT = ctx.enter_context(tc.tile_pool(name="pT", bufs=2))
        pdiff = ctx.enter_context(tc.tile_pool(name="pdiff", bufs=2))
        pacc = ctx.enter_context(tc.tile_pool(name="pacc", bufs=2))
        ppart = ctx.enter_context(tc.tile_pool(name="ppart", bufs=1))
        psA = ctx.enter_context(tc.tile_pool(name="psA", bufs=2,
                                             space="PSUM"))
        psB = ctx.enter_context(tc.tile_pool(name="psB", bufs=1,
                                             space="PSUM"))

        Wf1 = consts.tile([3, 128], F32R)
        nc.sync.dma_start(out=Wf1, in_=d_Wf1.bitcast(F32R))
        W23 = consts.tile([128, 128], F16)
        nc.sync.dma_start(out=W23, in_=d_W23)
        Wf2p = consts.tile([128, 32], F16)
        nc.sync.dma_start(out=Wf2p, in_=d_Wf2p)
        Wg2p = consts.tile([128, 32], F16)
        nc.sync.dma_start(out=Wg2p, in_=d_Wg2p)
        bf1 = consts.tile([128, 1], F32)
        nc.sync.dma_start(out=bf1, in_=d_bf1)
        b23 = consts.tile([128, 1], F32)
        nc.sync.dma_start(out=b23, in_=d_b23)
        ones = consts.tile([96, 1], F32)
        nc.vector.memset(ones[:], 1.0)

        partials = ppart.tile([1, NW * 288], F32)

        for w in range(NW):
            posT_w = pposT.tile([3, WINDOW], F32R, tag="posT")
            nc.sync.dma_start(
                out=posT_w,
                in_=d_posT[:, WINDOW * w:WINDOW * (w + 1)].bitcast(F32R))
            posb_w = pposb.tile([96, 96], F32, tag="posb")
            for t in range(2):
                nc.sync.dma_start(out=posb_w[:, 48 * t:48 * (t + 1)],
                                  in_=d_posb[2 * w + t])

            acc = pacc.tile([96, 288], F32, tag="acc")

            for t in range(2):
                triad = 2 * w + t
                aff3 = psA.tile([96, BLOCK], F32, tag="aff3")
                rec3 = psB.tile([96, BLOCK], F32, tag="rec3")

                for q in range(3):
                    blk = 3 * triad + q
                    cs = BLOCK * (3 * t + q)
                    h1 = psA.tile([128, BLOCK], F32, tag="h1")
                    nc.tensor.matmul(h1[:], Wf1[:],
                                     posT_w[:, cs:cs + BLOCK],
                                     start=True, stop=True)
                    s1 = ps1.tile([128, BLOCK], F16, tag="s1")
                    nc.scalar.activation(s1[:], h1[:], AF.Relu, bias=bf1[:])

                    pre2 = psA.tile([128, BLOCK], F32, tag="pre2")
                    nc.tensor.matmul(pre2[:], W23[:], s1[:],
                                     start=True, stop=True)
                    s2 = ps2.tile([128, BLOCK], F16, tag="s2")
                    if _relu2_on_act(blk):
                        nc.scalar.activation(s2[:], pre2[:], AF.Relu,
                                             bias=b23[:])
                    else:
                        nc.vector.tensor_scalar(s2[:], pre2[:], b23[:], 0.0,
                                                ALU.add, ALU.max)

                    nc.tensor.matmul(aff3[32 * q:32 * q + 32, :],
                                     Wf2p[:], s1[:], start=True, stop=True)
                    nc.tensor.matmul(rec3[32 * q:32 * q + 32, :],
                                     Wg2p[:], s2[:], start=True, stop=True)

                T_aff = pT.tile([96, BLOCK], F32, tag="Taff")
                nc.vector.transpose(T_aff[:], aff3[:])
                T_rec = pT.tile([96, BLOCK], F32, tag="Trec")
                nc.vector.transpose(T_rec[:], rec3[:])

                # aff per-point out: cols (j,a) a<16 of each 32-col group
                aff_src = T_aff[:, :].rearrange("p (j a) -> p j a",
                                                a=32)[:, :, 0:16]
                nc.sync.dma_start(out=d_aff[triad], in_=aff_src)

                # diff = posb - rec   (per-point layout, strided rec cols)
                rec_src = T_rec[:, :].rearrange("p (j c) -> p j c",
                                                c=32)[:, :, 0:3]
                diff = pdiff.tile([96, 48], F32, tag="diff")
                nc.vector.tensor_tensor(
                    diff[:], posb_w[:, 48 * t:48 * (t + 1)],
                    rec_src, ALU.subtract)
                nc.sync.dma_start(out=d_diff[triad], in_=diff[:])

                sq = pdiff.tile([96, 48], F32, tag="sq")
                nc.scalar.activation(sq[:], diff[:], AF.Square)
                nc.vector.reduce_sum(
                    acc[:, 256 + 16 * t:256 + 16 * (t + 1)],
                    sq[:, :].rearrange("p (j c) -> p j c", c=3),
                    axis=AX.X)

                if t == 0:
                    nc.vector.tensor_copy(acc[:, 0:256], aff_src)
                else:
                    nc.vector.tensor_tensor(acc[:, 0:256], acc[:, 0:256],
                                            aff_src, ALU.add)

            fl = psB.tile([1, 288], F32, tag="flush")
            nc.tensor.matmul(fl[0:1, :], ones[:], acc[:],
                             start=True, stop=True)
            nc.scalar.activation(partials[0:1, 288 * w:288 * (w + 1)],
                                 fl[0:1, :], AF.Copy)
            nc.sync.dma_start(out=d_errp[w], in_=acc[:, 256:288])

        nc.sync.dma_start(out=d_part, in_=partials[:])

    nc.compile()
    _CACHE["prog"] = nc
    return nc


def _host_prep(pos, bg2):
    """Per-core input arrays from the full pos."""
    in_maps = []
    for c in range(NCORES):
        chunk = np.zeros((NPAD, 3), np.float32)
        chunk[:NC] = pos[c * NC:(c + 1) * NC]
        posT = np.ascontiguousarray(chunk.T)
        posb = (chunk - bg2[None, :]).reshape(NTRIAD, 3, 16, 32, 3)
        posb = np.ascontiguousarray(posb.transpose(0, 1, 3, 2, 4)
                                    ).reshape(NTRIAD, 96, 48)
        in_maps.append({"posT": posT, "posb": posb})
    return in_maps


def _unperm_aff(aff_perm):
    # (NTRIAD, 96, 16, 16) [t, (q p), j, a] -> (NPAD, 16)
    a = aff_perm.reshape(NTRIAD, 3, 32, 16, 16).transpose(0, 1, 3, 2, 4)
    return np.ascontiguousarray(a).reshape(NPAD, 16)


def _unperm_diff(diff_perm):
    # (NTRIAD, 96, 48) [t, (q p), (j c)] -> (NPAD, 3)
    d = diff_perm.reshape(NTRIAD, 3, 32, 16, 3).transpose(0, 1, 3, 2, 4)
    return np.ascontiguousarray(d).reshape(NPAD, 3)


def _unperm_err(errp):
    # (NW, 96, 32) [w, (q p), (t j)] -> (NPAD,)
    e = errp.reshape(NW, 3, 32, 2, 16).transpose(0, 3, 1, 4, 2)
    return np.ascontiguousarray(e).reshape(NPAD)


def kernel(pos, batch, agent_h, coherence_signal_prev, coherence_spatial_prev,
           Wf1, bf1, Wf2, bf2, Wg1, bg1, Wg2, bg2,
           Wx, Wh, bx, bh, Wlat, blat, Wact, bact):
    pos = np.asarray(pos, np.float32)
    batch = np.asarray(batch, np.int32)
    agent_h = np.asarray(agent_h, np.float32)
    Wf1 = np.asarray(Wf1, np.float32)
    bf1 = np.asarray(bf1, np.float32)
    Wf2 = np.asarray(Wf2, np.float32)
    bf2 = np.asarray(bf2, np.float32)
    Wg1 = np.asarray(Wg1, np.float32)
    bg1 = np.asarray(bg1, np.float32)
    Wg2 = np.asarray(Wg2, np.float32)
    bg2 = np.asarray(bg2, np.float32)

    nc = _build_program()

    # folded middle matmul + deferred biases
    W23 = (Wf2.astype(np.float64) @ Wg1.astype(np.float64)).astype(np.float32)
    b23 = (bf2.astype(np.float64) @ Wg1.astype(np.float64)
           + bg1.astype(np.float64)).astype(np.float32)

    Wf2p = np.zeros((128, 32), np.float16)
    Wf2p[:, 0:16] = Wf2.astype(np.float16)
    Wg2p = np.zeros((128, 32), np.float16)
    Wg2p[:, 0:3] = Wg2.astype(np.float16)

    common = {
        "Wf1": np.ascontiguousarray(Wf1),
        "W23": W23.astype(np.float16),
        "Wf2p": Wf2p,
        "Wg2p": Wg2p,
        "bf1": np.ascontiguousarray(bf1.reshape(128, 1)),
        "b23": np.ascontiguousarray(b23.reshape(128, 1)),
    }
    in_maps = _host_prep(pos, bg2)
    for m in in_maps:
        m.update(common)

    res = run_bass_kernel_spmd(nc, in_maps, list(range(NCORES)))
    outs = res.results

    affordances = np.empty((N, 16), np.float32)
    reconstructed = np.empty((N, 3), np.float32)
    coherence_spatial = np.empty((N,), np.float32)

    seg_aff = np.zeros((B, 16), np.float64)
    seg_err = np.zeros((B,), np.float64)
    counts = np.bincount(batch, minlength=B).astype(np.float64)
    starts = np.searchsorted(batch, np.arange(B + 1))

    for c in range(NCORES):
        o = outs[c]
        aff0 = _unperm_aff(o["aff"])[:NC]
        diff = _unperm_diff(o["diff"])[:NC]
        err = _unperm_err(o["errp"])[:NC]
        lo = c * NC
        affordances[lo:lo + NC] = aff0 + bf2[None, :]
        reconstructed[lo:lo + NC] = pos[lo:lo + NC] - diff
        coherence_spatial[lo:lo + NC] = err

        parts = o["part"].reshape(NW, 288)
        aff_w = parts[:, 0:256].reshape(NW, 16, 16).sum(axis=1)
        err_w = parts[:, 256:288].sum(axis=1)

        for w in range(NW):
            g0 = lo + w * WINDOW
            g1 = min(g0 + WINDOW, lo + NC)
            s_lo = batch[g0]
            s_hi = batch[g1 - 1]
            full = (g1 - g0) == WINDOW
            if full and s_lo == s_hi:
                seg_aff[s_lo] += aff_w[w].astype(np.float64)
                seg_err[s_lo] += float(err_w[w])
            else:
                for s in range(s_lo, s_hi + 1):
                    a = max(g0, starts[s])
                    b_ = min(g1, starts[s + 1])
                    if b_ > a:
                        seg_aff[s] += aff0[a - lo:b_ - lo].sum(
                            axis=0, dtype=np.float64)
                        seg_err[s] += err[a - lo:b_ - lo].sum(
                            dtype=np.float64)

    denom = np.maximum(counts, 1.0)
    coherence_signal = (seg_err / denom).astype(np.float32)[:, None]
    batch_aff = (seg_aff / denom[:, None]).astype(np.float32) + bf2[None, :]

    # tiny GRU + heads on host (B=64)
    Wx = np.asarray(Wx, np.float32)
    Wh = np.asarray(Wh, np.float32)
    bx = np.asarray(bx, np.float32)
    bh = np.asarray(bh, np.float32)
    Wlat = np.asarray(Wlat, np.float32)
    blat = np.asarray(blat, np.float32)
    Wact = np.asarray(Wact, np.float32)
    bact = np.asarray(bact, np.float32)

    gx = batch_aff @ Wx + bx
    gh = agent_h @ Wh + bh
    AH = agent_h.shape[1]
    gx_r, gx_z, gx_n = gx[:, :AH], gx[:, AH:2 * AH], gx[:, 2 * AH:]
    gh_r, gh_z, gh_n = gh[:, :AH], gh[:, AH:2 * AH], gh[:, 2 * AH:]

    def sigmoid(v):
        return 1.0 / (1.0 + np.exp(-v))

    r = sigmoid(gx_r + gh_r)
    z = sigmoid(gx_z + gh_z)
    n_ = np.tanh(gx_n + r * gh_n)
    agent_h_next = (1.0 - z) * n_ + z * agent_h
    latent = np.tanh(agent_h_next @ Wlat + blat)
    agent_action = latent @ Wact + bact

    return (affordances, reconstructed, coherence_signal.astype(np.float32),
            coherence_spatial, agent_action.astype(np.float32),
            agent_h_next.astype(np.float32))


# revision 5
# speedup vs baseline: 1.0012x; 1.0012x over previous
"""Trainium2 Bass kernel for nn_AdjunctionModel (segment_reduce).

Math (per point, N=1e6 points, B=64 sorted segments):
    h1   = relu(pos @ Wf1 + bf1)            (N,128)
    aff  = h1 @ Wf2 + bf2                   (N,16)   [output]
    h2   = relu(aff @ Wg1 + bg1)            (N,128)
    rec  = h2 @ Wg2 + bg2                   (N,3)    [output]
    err  = sum((pos - rec)^2, -1)           (N,)     [output]
    per-segment means of err and aff feed a tiny GRU (B=64).

Key algebraic fold: there is no nonlinearity between the two middle
matmuls, so  h2 = relu(h1 @ (Wf2 @ Wg1) + (bf2 @ Wg1 + bg1)).  The
device computes, per 512-point block:
    L1   : h1 = Wf1^T @ posT            (fp32r matmul, N=512)
    relu1: s1 = relu(h1 + bf1)          (ACT, fp16 out)
    L3   : pre2 = W23^T @ s1            (fp16 matmul)
    relu2: s2 = relu(pre2 + b23)        (ACT or DVE, alternating)
    aff0 : Wf2p^T @ s1  -> quadrant 32q of a psum tile   (fp16)
    rec0 : Wg2p^T @ s2  -> quadrant 32q of a second tile (fp16)
Per triad (3 blocks), a DVE 32x32 StreamTranspose turns the quadrant-
packed (96,512) psum tiles into per-point-layout and the per-window
(2 triads) partial sums are reduced by a ones-matmul.  Host combines
per-window partials into per-segment sums (recomputing the few windows
that straddle a segment boundary from the per-point outputs), adds the
deferred biases, and runs the tiny GRU in numpy.

Sharding: data-parallel over points, 8 cores, same NEFF on every core
(per-core inputs differ only in data).
"""

import os
import sys
from contextlib import ExitStack

import numpy as np

sys.path.insert(0, "/opt/trn_rl_repo")

import ml_dtypes  # noqa: E402
import concourse.bass as bass  # noqa: E402
import concourse.tile as tile  # noqa: E402
from concourse import bacc, mybir  # noqa: E402
from concourse.bass_utils import run_bass_kernel_spmd  # noqa: E402

F32 = mybir.dt.float32
F32R = mybir.dt.float32r
F16 = mybir.dt.float16
BF16 = mybir.dt.bfloat16
HID_DT = BF16 if os.environ.get("KHID", "bf16") == "bf16" else F16
AF = mybir.ActivationFunctionType
ALU = mybir.AluOpType
AX = mybir.AxisListType

N = 1_000_000
B = 64
NCORES = 8
NC = N // NCORES           # 125000 points per core
BLOCK = 512
TRIAD = 3 * BLOCK          # 1536
WINDOW = 2 * TRIAD         # 3072
NW = (NC + WINDOW - 1) // WINDOW   # 41
NPAD = NW * WINDOW         # 125952
NTRIAD = 2 * NW            # 82

# relu2 engine assignment: ACT on even blocks, DVE on odd (≈0.5 split)
def _relu2_on_act(blk: int) -> bool:
    return blk % 2 == 0


_CACHE = {}


def _build_program():
    if "prog" in _CACHE:
        return _CACHE["prog"]

    nc = bacc.Bacc("TRN2", target_bir_lowering=False, debug=False,
                   num_devices=NCORES)

    # ---- DRAM I/O ----
    d_posT = nc.dram_tensor("posT", [3, NPAD], F32, kind="ExternalInput").ap()
    d_posb = nc.dram_tensor("posb", [NTRIAD, 96, 48], F32,
                            kind="ExternalInput").ap()
    d_Wf1 = nc.dram_tensor("Wf1", [3, 128], F32, kind="ExternalInput").ap()
    d_W23 = nc.dram_tensor("W23", [128, 128], HID_DT, kind="ExternalInput").ap()
    d_Wf2p = nc.dram_tensor("Wf2p", [128, 32], HID_DT, kind="ExternalInput").ap()
    d_Wg2p = nc.dram_tensor("Wg2p", [128, 32], HID_DT, kind="ExternalInput").ap()
    d_bf1 = nc.dram_tensor("bf1", [128, 1], F32, kind="ExternalInput").ap()
    d_b23 = nc.dram_tensor("b23", [128, 1], F32, kind="ExternalInput").ap()

    d_aff = nc.dram_tensor("aff", [NTRIAD, 96, 16, 16], F32,
                           kind="ExternalOutput").ap()
    d_diff = nc.dram_tensor("diff", [NTRIAD, 96, 48], F32,
                            kind="ExternalOutput").ap()
    d_errp = nc.dram_tensor("errp", [NW, 96, 32], F32,
                            kind="ExternalOutput").ap()
    d_part = nc.dram_tensor("part", [1, NW * 288], F32,
                            kind="ExternalOutput").ap()

    with tile.TileContext(nc) as tc, ExitStack() as ctx:
        consts = ctx.enter_context(tc.tile_pool(name="consts", bufs=1))
        pposT = ctx.enter_context(tc.tile_pool(name="pposT", bufs=2))
        pposb = ctx.enter_context(tc.tile_pool(name="pposb", bufs=2))
        ps1 = ctx.enter_context(tc.tile_pool(name="ps1", bufs=2))
        ps2 = ctx.enter_context(tc.tile_pool(name="ps2", bufs=2))
        pT = ctx.enter_context(tc.tile_pool(name="pT", bufs=2))
        pdiff = ctx.enter_context(tc.tile_pool(name="pdiff", bufs=2))
        pacc = ctx.enter_context(tc.tile_pool(name="pacc", bufs=2))
        ppart = ctx.enter_context(tc.tile_pool(name="ppart", bufs=1))
        psA = ctx.enter_context(tc.tile_pool(name="psA", bufs=2,
                                             space="PSUM"))
        psB = ctx.enter_context(tc.tile_pool(name="psB", bufs=1,
                                             space="PSUM"))

        Wf1 = consts.tile([3, 128], F32R)
        nc.sync.dma_start(out=Wf1, in_=d_Wf1.bitcast(F32R))
        W23 = consts.tile([128, 128], HID_DT)
        nc.sync.dma_start(out=W23, in_=d_W23)
        Wf2p = consts.tile([128, 32], HID_DT)
        nc.sync.dma_start(out=Wf2p, in_=d_Wf2p)
        Wg2p = consts.tile([128, 32], HID_DT)
        nc.sync.dma_start(out=Wg2p, in_=d_Wg2p)
        bf1 = consts.tile([128, 1], F32)
        nc.sync.dma_start(out=bf1, in_=d_bf1)
        b23 = consts.tile([128, 1], F32)
        nc.sync.dma_start(out=b23, in_=d_b23)
        ones = consts.tile([96, 1], F32)
        nc.vector.memset(ones[:], 1.0)

        partials = ppart.tile([1, NW * 288], F32)

        for w in range(NW):
            posT_w = pposT.tile([3, WINDOW], F32R, tag="posT")
            nc.sync.dma_start(
                out=posT_w,
                in_=d_posT[:, WINDOW * w:WINDOW * (w + 1)].bitcast(F32R))
            posb_w = pposb.tile([96, 96], F32, tag="posb")
            for t in range(2):
                nc.sync.dma_start(out=posb_w[:, 48 * t:48 * (t + 1)],
                                  in_=d_posb[2 * w + t])

            acc = pacc.tile([96, 288], F32, tag="acc")

            for t in range(2):
                triad = 2 * w + t
                aff3 = psA.tile([96, BLOCK], F32, tag="aff3")
                rec3 = psB.tile([96, BLOCK], F32, tag="rec3")

                for q in range(3):
                    blk = 3 * triad + q
                    cs = BLOCK * (3 * t + q)
                    h1 = psA.tile([128, BLOCK], F32, tag="h1")
                    nc.tensor.matmul(h1[:], Wf1[:],
                                     posT_w[:, cs:cs + BLOCK],
                                     start=True, stop=True)
                    s1 = ps1.tile([128, BLOCK], HID_DT, tag="s1")
                    nc.scalar.activation(s1[:], h1[:], AF.Relu, bias=bf1[:])

                    pre2 = psA.tile([128, BLOCK], F32, tag="pre2")
                    nc.tensor.matmul(pre2[:], W23[:], s1[:],
                                     start=True, stop=True)
                    s2 = ps2.tile([128, BLOCK], HID_DT, tag="s2")
                    if _relu2_on_act(blk):
                        nc.scalar.activation(s2[:], pre2[:], AF.Relu,
                                             bias=b23[:])
                    else:
                        nc.vector.tensor_scalar(s2[:], pre2[:], b23[:], 0.0,
                                                ALU.add, ALU.max)

                    nc.tensor.matmul(aff3[32 * q:32 * q + 32, :],
                                     Wf2p[:], s1[:], start=True, stop=True)
                    nc.tensor.matmul(rec3[32 * q:32 * q + 32, :],
                                     Wg2p[:], s2[:], start=True, stop=True)

                T_aff = pT.tile([96, BLOCK], F32, tag="Taff")
                nc.vector.transpose(T_aff[:], aff3[:])
                T_rec = pT.tile([96, BLOCK], F32, tag="Trec")
                nc.vector.transpose(T_rec[:], rec3[:])

                # aff per-point out: cols (j,a) a<16 of each 32-col group
                aff_src = T_aff[:, :].rearrange("p (j a) -> p j a",
                                                a=32)[:, :, 0:16]
                nc.sync.dma_start(out=d_aff[triad], in_=aff_src)

                # diff = posb - rec   (per-point layout, strided rec cols)
                rec_src = T_rec[:, :].rearrange("p (j c) -> p j c",
                                                c=32)[:, :, 0:3]
                diff = pdiff.tile([96, 48], F32, tag="diff")
                nc.vector.tensor_tensor(
                    diff[:], posb_w[:, 48 * t:48 * (t + 1)],
                    rec_src, ALU.subtract)
                nc.sync.dma_start(out=d_diff[triad], in_=diff[:])

                sq = pdiff.tile([96, 48], F32, tag="sq")
                nc.scalar.activation(sq[:], diff[:], AF.Square)
                nc.vector.reduce_sum(
                    acc[:, 256 + 16 * t:256 + 16 * (t + 1)],
                    sq[:, :].rearrange("p (j c) -> p j c", c=3),
                    axis=AX.X)

                if t == 0:
                    nc.vector.tensor_copy(acc[:, 0:256], aff_src)
                else:
                    nc.vector.tensor_tensor(acc[:, 0:256], acc[:, 0:256],
                                            aff_src, ALU.add)

            fl = psB.tile([1, 288], F32, tag="flush")
            nc.tensor.matmul(fl[0:1, :], ones[:], acc[:],
                             start=True, stop=True)
            nc.scalar.activation(partials[0:1, 288 * w:288 * (w + 1)],
                                 fl[0:1, :], AF.Copy)
            nc.sync.dma_start(out=d_errp[w], in_=acc[:, 256:288])

        nc.sync.dma_start(out=d_part, in_=partials[:])

    nc.compile()
    _CACHE["prog"] = nc
    return nc


def _host_prep(pos, bg2):
    """Per-core input arrays from the full pos."""
    in_maps = []
    for c in range(NCORES):
        chunk = np.zeros((NPAD, 3), np.float32)
        chunk[:NC] = pos[c * NC:(c + 1) * NC]
        posT = np.ascontiguousarray(chunk.T)
        posb = (chunk - bg2[None, :]).reshape(NTRIAD, 3, 16, 32, 3)
        posb = np.ascontiguousarray(posb.transpose(0, 1, 3, 2, 4)
                                    ).reshape(NTRIAD, 96, 48)
        in_maps.append({"posT": posT, "posb": posb})
    return in_maps


def _unperm_aff(aff_perm):
    # (NTRIAD, 96, 16, 16) [t, (q p), j, a] -> (NPAD, 16)
    a = aff_perm.reshape(NTRIAD, 3, 32, 16, 16).transpose(0, 1, 3, 2, 4)
    return np.ascontiguousarray(a).reshape(NPAD, 16)


def _unperm_diff(diff_perm):
    # (NTRIAD, 96, 48) [t, (q p), (j c)] -> (NPAD, 3)
    d = diff_perm.reshape(NTRIAD, 3, 32, 16, 3).transpose(0, 1, 3, 2, 4)
    return np.ascontiguousarray(d).reshape(NPAD, 3)


def _unperm_err(errp):
    # (NW, 96, 32) [w, (q p), (t j)] -> (NPAD,)
    e = errp.reshape(NW, 3, 32, 2, 16).transpose(0, 3, 1, 4, 2)
    return np.ascontiguousarray(e).reshape(NPAD)




def _make_in_maps(pos, Wf1, bf1, Wf2, bf2, Wg1, bg1, Wg2, bg2):
    # folded middle matmul + deferred biases
    W23 = (Wf2.astype(np.float64) @ Wg1.astype(np.float64)).astype(np.float32)
    b23 = (bf2.astype(np.float64) @ Wg1.astype(np.float64)
           + bg1.astype(np.float64)).astype(np.float32)

    np_hid = ml_dtypes.bfloat16 if os.environ.get("KHID", "bf16") == "bf16" \
        else np.float16
    Wf2p = np.zeros((128, 32), np_hid)
    Wf2p[:, 0:16] = Wf2.astype(np_hid)
    Wg2p = np.zeros((128, 32), np_hid)
    Wg2p[:, 0:3] = Wg2.astype(np_hid)

    common = {
        "Wf1": np.ascontiguousarray(Wf1),
        "W23": W23.astype(np_hid),
        "Wf2p": Wf2p,
        "Wg2p": Wg2p,
        "bf1": np.ascontiguousarray(bf1.reshape(128, 1)),
        "b23": np.ascontiguousarray(b23.reshape(128, 1)),
    }
    in_maps = _host_prep(pos, bg2)
    for m in in_maps:
        m.update(common)
    return in_maps




def kernel(pos, batch, agent_h, coherence_signal_prev, coherence_spatial_prev,
           Wf1, bf1, Wf2, bf2, Wg1, bg1, Wg2, bg2,
           Wx, Wh, bx, bh, Wlat, blat, Wact, bact):
    pos = np.asarray(pos, np.float32)
    batch = np.asarray(batch, np.int32)
    agent_h = np.asarray(agent_h, np.float32)
    Wf1 = np.asarray(Wf1, np.float32)
    bf1 = np.asarray(bf1, np.float32)
    Wf2 = np.asarray(Wf2, np.float32)
    bf2 = np.asarray(bf2, np.float32)
    Wg1 = np.asarray(Wg1, np.float32)
    bg1 = np.asarray(bg1, np.float32)
    Wg2 = np.asarray(Wg2, np.float32)
    bg2 = np.asarray(bg2, np.float32)

    nc = _build_program()
    in_maps = _make_in_maps(pos, Wf1, bf1, Wf2, bf2, Wg1, bg1, Wg2, bg2)

    res = run_bass_kernel_spmd(nc, in_maps, list(range(NCORES)))
    outs = res.results

    affordances = np.empty((N, 16), np.float32)
    reconstructed = np.empty((N, 3), np.float32)
    coherence_spatial = np.empty((N,), np.float32)

    seg_aff = np.zeros((B, 16), np.float64)
    seg_err = np.zeros((B,), np.float64)
    counts = np.bincount(batch, minlength=B).astype(np.float64)
    starts = np.searchsorted(batch, np.arange(B + 1))

    for c in range(NCORES):
        o = outs[c]
        aff0 = _unperm_aff(o["aff"])[:NC]
        diff = _unperm_diff(o["diff"])[:NC]
        err = _unperm_err(o["errp"])[:NC]
        lo = c * NC
        affordances[lo:lo + NC] = aff0 + bf2[None, :]
        reconstructed[lo:lo + NC] = pos[lo:lo + NC] - diff
        coherence_spatial[lo:lo + NC] = err

        parts = o["part"].reshape(NW, 288)
        aff_w = parts[:, 0:256].reshape(NW, 16, 16).sum(axis=1)
        err_w = parts[:, 256:288].sum(axis=1)

        for w in range(NW):
            g0 = lo + w * WINDOW
            g1 = min(g0 + WINDOW, lo + NC)
            s_lo = batch[g0]
            s_hi = batch[g1 - 1]
            full = (g1 - g0) == WINDOW
            if full and s_lo == s_hi:
                seg_aff[s_lo] += aff_w[w].astype(np.float64)
                seg_err[s_lo] += float(err_w[w])
            else:
                for s in range(s_lo, s_hi + 1):
                    a = max(g0, starts[s])
                    b_ = min(g1, starts[s + 1])
                    if b_ > a:
                        seg_aff[s] += aff0[a - lo:b_ - lo].sum(
                            axis=0, dtype=np.float64)
                        seg_err[s] += err[a - lo:b_ - lo].sum(
                            dtype=np.float64)

    denom = np.maximum(counts, 1.0)
    coherence_signal = (seg_err / denom).astype(np.float32)[:, None]
    batch_aff = (seg_aff / denom[:, None]).astype(np.float32) + bf2[None, :]

    # tiny GRU + heads on host (B=64)
    Wx = np.asarray(Wx, np.float32)
    Wh = np.asarray(Wh, np.float32)
    bx = np.asarray(bx, np.float32)
    bh = np.asarray(bh, np.float32)
    Wlat = np.asarray(Wlat, np.float32)
    blat = np.asarray(blat, np.float32)
    Wact = np.asarray(Wact, np.float32)
    bact = np.asarray(bact, np.float32)

    gx = batch_aff @ Wx + bx
    gh = agent_h @ Wh + bh
    AH = agent_h.shape[1]
    gx_r, gx_z, gx_n = gx[:, :AH], gx[:, AH:2 * AH], gx[:, 2 * AH:]
    gh_r, gh_z, gh_n = gh[:, :AH], gh[:, AH:2 * AH], gh[:, 2 * AH:]

    def sigmoid(v):
        return 1.0 / (1.0 + np.exp(-v))

    r = sigmoid(gx_r + gh_r)
    z = sigmoid(gx_z + gh_z)
    n_ = np.tanh(gx_n + r * gh_n)
    agent_h_next = (1.0 - z) * n_ + z * agent_h
    latent = np.tanh(agent_h_next @ Wlat + blat)
    agent_action = latent @ Wact + bact

    return (affordances, reconstructed, coherence_signal.astype(np.float32),
            coherence_spatial, agent_action.astype(np.float32),
            agent_h_next.astype(np.float32))


# revision 8
# speedup vs baseline: 1.0948x; 1.0935x over previous
"""Trainium2 Bass kernel for nn_AdjunctionModel (segment_reduce).

Math (per point, N=1e6 points, B=64 sorted segments):
    h1   = relu(pos @ Wf1 + bf1)            (N,128)
    aff  = h1 @ Wf2 + bf2                   (N,16)   [output]
    h2   = relu(aff @ Wg1 + bg1)            (N,128)
    rec  = h2 @ Wg2 + bg2                   (N,3)    [output]
    err  = sum((pos - rec)^2, -1)           (N,)     [output]
    per-segment means of err and aff feed a tiny GRU (B=64).

Key algebraic fold: there is no nonlinearity between the two middle
matmuls, so  h2 = relu(h1 @ (Wf2 @ Wg1) + (bf2 @ Wg1 + bg1)).  The
device computes, per 512-point block:
    L1   : h1 = Wf1^T @ posT            (fp32r matmul, N=512)
    relu1: s1 = relu(h1 + bf1)          (ACT, fp16 out)
    L3   : pre2 = W23^T @ s1            (fp16 matmul)
    relu2: s2 = relu(pre2 + b23)        (ACT or DVE, alternating)
    aff0 : Wf2p^T @ s1  -> quadrant 32q of a psum tile   (fp16)
    rec0 : Wg2p^T @ s2  -> quadrant 32q of a second tile (fp16)
Per triad (3 blocks), a DVE 32x32 StreamTranspose turns the quadrant-
packed (96,512) psum tiles into per-point-layout and the per-window
(2 triads) partial sums are reduced by a ones-matmul.  Host combines
per-window partials into per-segment sums (recomputing the few windows
that straddle a segment boundary from the per-point outputs), adds the
deferred biases, and runs the tiny GRU in numpy.

Sharding: data-parallel over points, 8 cores, same NEFF on every core
(per-core inputs differ only in data).
"""

import os
import sys
from contextlib import ExitStack

import numpy as np

sys.path.insert(0, "/opt/trn_rl_repo")

import ml_dtypes  # noqa: E402
import concourse.bass as bass  # noqa: E402
import concourse.tile as tile  # noqa: E402
from concourse import bacc, mybir  # noqa: E402
from concourse.bass_utils import run_bass_kernel_spmd  # noqa: E402

F32 = mybir.dt.float32
F32R = mybir.dt.float32r
F16 = mybir.dt.float16
BF16 = mybir.dt.bfloat16
HID_DT = BF16 if os.environ.get("KHID", "bf16") == "bf16" else F16
AF = mybir.ActivationFunctionType
ALU = mybir.AluOpType
AX = mybir.AxisListType

N = 1_000_000
B = 64
NCORES = 8
NC = N // NCORES           # 125000 points per core
BLOCK = 512
TRIAD = 3 * BLOCK          # 1536
WINDOW = 2 * TRIAD         # 3072
NW = (NC + WINDOW - 1) // WINDOW   # 41
NPAD = NW * WINDOW         # 125952
NTRIAD = 2 * NW            # 82

# relu2 engine assignment: ACT on even blocks, DVE on odd (≈0.5 split)
def _relu2_on_act(blk: int) -> bool:
    return blk % 2 == 0


_CACHE = {}


def _build_program():
    if "prog" in _CACHE:
        return _CACHE["prog"]

    nc = bacc.Bacc("TRN2", target_bir_lowering=False, debug=False,
                   num_devices=NCORES)

    # ---- DRAM I/O ----
    d_posT = nc.dram_tensor("posT", [3, NPAD], F32, kind="ExternalInput").ap()
    d_posb = nc.dram_tensor("posb", [NTRIAD, 96, 48], F32,
                            kind="ExternalInput").ap()
    d_Wf1 = nc.dram_tensor("Wf1", [3, 128], F32, kind="ExternalInput").ap()
    d_W23 = nc.dram_tensor("W23", [128, 128], HID_DT, kind="ExternalInput").ap()
    d_Wf2p = nc.dram_tensor("Wf2p", [128, 32], HID_DT, kind="ExternalInput").ap()
    d_Wg2p = nc.dram_tensor("Wg2p", [128, 32], HID_DT, kind="ExternalInput").ap()
    d_bf1 = nc.dram_tensor("bf1", [128, 1], F32, kind="ExternalInput").ap()
    d_b23 = nc.dram_tensor("b23", [128, 1], F32, kind="ExternalInput").ap()

    d_aff = nc.dram_tensor("aff", [NTRIAD, 96, 16, 16], F32,
                           kind="ExternalOutput").ap()
    d_diff = nc.dram_tensor("diff", [NTRIAD, 96, 48], F32,
                            kind="ExternalOutput").ap()
    d_errp = nc.dram_tensor("errp", [NW, 96, 32], F32,
                            kind="ExternalOutput").ap()
    d_part = nc.dram_tensor("part", [1, NW * 288], F32,
                            kind="ExternalOutput").ap()

    with tile.TileContext(nc) as tc, ExitStack() as ctx:
        consts = ctx.enter_context(tc.tile_pool(name="consts", bufs=1))
        pposT = ctx.enter_context(tc.tile_pool(name="pposT", bufs=3))
        pposb = ctx.enter_context(tc.tile_pool(name="pposb", bufs=3))
        ps1 = ctx.enter_context(tc.tile_pool(name="ps1", bufs=3))
        ps2 = ctx.enter_context(tc.tile_pool(name="ps2", bufs=3))
        pT = ctx.enter_context(tc.tile_pool(name="pT", bufs=2))
        pdiff = ctx.enter_context(tc.tile_pool(name="pdiff", bufs=2))
        pacc = ctx.enter_context(tc.tile_pool(name="pacc", bufs=3))
        ppart = ctx.enter_context(tc.tile_pool(name="ppart", bufs=1))
        psAh = ctx.enter_context(tc.tile_pool(name="psAh", bufs=2,
                                              space="PSUM"))
        psA = ctx.enter_context(tc.tile_pool(name="psA", bufs=2,
                                             space="PSUM"))
        psB = ctx.enter_context(tc.tile_pool(name="psB", bufs=1,
                                             space="PSUM"))

        Wf1 = consts.tile([3, 128], F32R)
        nc.sync.dma_start(out=Wf1, in_=d_Wf1.bitcast(F32R))
        W23 = consts.tile([128, 128], HID_DT)
        nc.sync.dma_start(out=W23, in_=d_W23)
        Wf2p = consts.tile([128, 32], HID_DT)
        nc.sync.dma_start(out=Wf2p, in_=d_Wf2p)
        Wg2p = consts.tile([128, 32], HID_DT)
        nc.sync.dma_start(out=Wg2p, in_=d_Wg2p)
        bf1 = consts.tile([128, 1], F32)
        nc.sync.dma_start(out=bf1, in_=d_bf1)
        b23 = consts.tile([128, 1], F32)
        nc.sync.dma_start(out=b23, in_=d_b23)
        ones = consts.tile([96, 1], F32)
        nc.vector.memset(ones[:], 1.0)

        partials = ppart.tile([1, NW * 288], F32)

        NB = 3 * NTRIAD  # 246 blocks
        # Software pipeline over "slots" so every PE matmul's input was
        # produced >=1 slot earlier (keeps PE dense -> HAM stays warm):
        #   slot i: L1(i) | L3'(i-1), aff0(i-1) | rec(i-2)
        state = {}          # per-block tiles: posT_w ref, s1, s2
        triad_state = {}    # per-triad: aff3, rec3, T_aff
        win_state = {}      # per-window: posT_w, posb_w, acc

        def load_window(w):
            posT_w = pposT.tile([3, WINDOW], F32R, tag="posT")
            nc.sync.dma_start(
                out=posT_w,
                in_=d_posT[:, WINDOW * w:WINDOW * (w + 1)].bitcast(F32R))
            posb_w = pposb.tile([96, 96], F32, tag="posb")
            for t in range(2):
                nc.sync.dma_start(out=posb_w[:, 48 * t:48 * (t + 1)],
                                  in_=d_posb[2 * w + t])
            acc = pacc.tile([96, 288], F32, tag="acc")
            win_state[w] = (posT_w, posb_w, acc)

        def front(i):
            # L1(i) + relu1(i); prefetch next window's inputs
            w = i // 6
            for ww in (w, w + 1):
                if ww < NW and ww not in win_state:
                    load_window(ww)
            posT_w, _, _ = win_state[w]
            cs = BLOCK * (i % 6)
            h1 = psAh.tile([128, BLOCK], F32, tag="h1")
            nc.tensor.matmul(h1[:], Wf1[:], posT_w[:, cs:cs + BLOCK],
                             start=True, stop=True)
            s1 = ps1.tile([128, BLOCK], HID_DT, tag="s1")
            nc.scalar.activation(s1[:], h1[:], AF.Relu, bias=bf1[:])
            state[i] = {"s1": s1}

        def mid(i):
            # L3'(i) + relu2(i) + aff0(i)  (inputs from slot i, ready)
            s1 = state[i]["s1"]
            t3 = i // 3
            if t3 not in triad_state:
                triad_state[t3] = {
                    "aff3": psA.tile([96, BLOCK], F32, name="aff3", tag="aff3"),
                    "rec3": psB.tile([96, BLOCK], F32, name="rec3", tag="rec3"),
                }
            q = i % 3
            pre2 = psA.tile([128, BLOCK], F32, tag="pre2")
            nc.tensor.matmul(pre2[:], W23[:], s1[:], start=True, stop=True)
            nc.tensor.matmul(triad_state[t3]["aff3"][32 * q:32 * q + 32, :],
                             Wf2p[:], s1[:], start=True, stop=True)
            s2 = ps2.tile([128, BLOCK], HID_DT, tag="s2")
            if _relu2_on_act(i):
                nc.scalar.activation(s2[:], pre2[:], AF.Relu, bias=b23[:])
            else:
                nc.vector.tensor_scalar(s2[:], pre2[:], b23[:], 0.0,
                                        ALU.add, ALU.max)
            state[i]["s2"] = s2

        def rec(i):
            # rec(i) matmul (s2 produced a slot earlier)
            t3, q = i // 3, i % 3
            nc.tensor.matmul(triad_state[t3]["rec3"][32 * q:32 * q + 32, :],
                             Wg2p[:], state[i]["s2"], start=True, stop=True)
            del state[i]

        def aff_tail(t3):
            # after aff3(t3) fully written: transpose, DMA aff, acc aff part
            w, t = t3 // 2, t3 % 2
            acc = win_state[w][2]
            T_aff = pT.tile([96, BLOCK], F32, tag="Taff")
            nc.vector.transpose(T_aff[:], triad_state[t3]["aff3"][:])
            aff_src = T_aff[:, :].rearrange("p (j a) -> p j a",
                                            a=32)[:, :, 0:16]
            nc.sync.dma_start(out=d_aff[t3], in_=aff_src)
            if t == 0:
                nc.vector.tensor_copy(acc[:, 0:256], aff_src)
            else:
                nc.vector.tensor_tensor(acc[:, 0:256], acc[:, 0:256],
                                        aff_src, ALU.add)

        def rec_tail(t3):
            # after rec3(t3) fully written: transpose, diff, sq, err reduce
            w, t = t3 // 2, t3 % 2
            posb_w, acc = win_state[w][1], win_state[w][2]
            T_rec = pT.tile([96, BLOCK], F32, tag="Trec")
            nc.vector.transpose(T_rec[:], triad_state[t3]["rec3"][:])
            rec_src = T_rec[:, :].rearrange("p (j c) -> p j c",
                                            c=32)[:, :, 0:3]
            diff = pdiff.tile([96, 48], F32, tag="diff")
            nc.vector.tensor_tensor(diff[:], posb_w[:, 48 * t:48 * (t + 1)],
                                    rec_src, ALU.subtract)
            nc.sync.dma_start(out=d_diff[t3], in_=diff[:])
            sq = pdiff.tile([96, 48], F32, tag="sq")
            nc.scalar.activation(sq[:], diff[:], AF.Square)
            nc.vector.reduce_sum(
                acc[:, 256 + 16 * t:256 + 16 * (t + 1)],
                sq[:, :].rearrange("p (j c) -> p j c", c=3),
                axis=AX.X)
            del triad_state[t3]

        def win_tail(w):
            # acc(w) complete: flush matmul + partials copy + errp DMA
            acc = win_state[w][2]
            fl = psB.tile([1, 288], F32, tag="flush")
            nc.tensor.matmul(fl[0:1, :], ones[:], acc[:],
                             start=True, stop=True)
            nc.scalar.activation(partials[0:1, 288 * w:288 * (w + 1)],
                                 fl[0:1, :], AF.Copy)
            nc.sync.dma_start(out=d_errp[w], in_=acc[:, 256:288])
            del win_state[w]

        for i in range(NB + 2):
            if i < NB:
                front(i)
            if 1 <= i <= NB:
                mid(i - 1)
                if (i - 1) % 3 == 2:
                    aff_tail((i - 1) // 3)
            if 2 <= i <= NB + 1:
                rec(i - 2)
                if (i - 2) % 3 == 2:
                    rec_tail((i - 2) // 3)
                    if ((i - 2) // 3) % 2 == 1:
                        win_tail((i - 2) // 6)

        nc.sync.dma_start(out=d_part, in_=partials[:])

    nc.compile()
    _CACHE["prog"] = nc
    return nc


def _host_prep(pos, bg2):
    """Per-core input arrays from the full pos."""
    in_maps = []
    for c in range(NCORES):
        chunk = np.zeros((NPAD, 3), np.float32)
        chunk[:NC] = pos[c * NC:(c + 1) * NC]
        posT = np.ascontiguousarray(chunk.T)
        posb = (chunk - bg2[None, :]).reshape(NTRIAD, 3, 16, 32, 3)
        posb = np.ascontiguousarray(posb.transpose(0, 1, 3, 2, 4)
                                    ).reshape(NTRIAD, 96, 48)
        in_maps.append({"posT": posT, "posb": posb})
    return in_maps


def _unperm_aff(aff_perm):
    # (NTRIAD, 96, 16, 16) [t, (q p), j, a] -> (NPAD, 16)
    a = aff_perm.reshape(NTRIAD, 3, 32, 16, 16).transpose(0, 1, 3, 2, 4)
    return np.ascontiguousarray(a).reshape(NPAD, 16)


def _unperm_diff(diff_perm):
    # (NTRIAD, 96, 48) [t, (q p), (j c)] -> (NPAD, 3)
    d = diff_perm.reshape(NTRIAD, 3, 32, 16, 3).transpose(0, 1, 3, 2, 4)
    return np.ascontiguousarray(d).reshape(NPAD, 3)


def _unperm_err(errp):
    # (NW, 96, 32) [w, (q p), (t j)] -> (NPAD,)
    e = errp.reshape(NW, 3, 32, 2, 16).transpose(0, 3, 1, 4, 2)
    return np.ascontiguousarray(e).reshape(NPAD)




def _make_in_maps(pos, Wf1, bf1, Wf2, bf2, Wg1, bg1, Wg2, bg2):
    # folded middle matmul + deferred biases
    W23 = (Wf2.astype(np.float64) @ Wg1.astype(np.float64)).astype(np.float32)
    b23 = (bf2.astype(np.float64) @ Wg1.astype(np.float64)
           + bg1.astype(np.float64)).astype(np.float32)

    np_hid = ml_dtypes.bfloat16 if os.environ.get("KHID", "bf16") == "bf16" \
        else np.float16
    Wf2p = np.zeros((128, 32), np_hid)
    Wf2p[:, 0:16] = Wf2.astype(np_hid)
    Wg2p = np.zeros((128, 32), np_hid)
    Wg2p[:, 0:3] = Wg2.astype(np_hid)

    common = {
        "Wf1": np.ascontiguousarray(Wf1),
        "W23": W23.astype(np_hid),
        "Wf2p": Wf2p,
        "Wg2p": Wg2p,
        "bf1": np.ascontiguousarray(bf1.reshape(128, 1)),
        "b23": np.ascontiguousarray(b23.reshape(128, 1)),
    }
    in_maps = _host_prep(pos, bg2)
    for m in in_maps:
        m.update(common)
    return in_maps




def kernel(pos, batch, agent_h, coherence_signal_prev, coherence_spatial_prev,
           Wf1, bf1, Wf2, bf2, Wg1, bg1, Wg2, bg2,
           Wx, Wh, bx, bh, Wlat, blat, Wact, bact):
    pos = np.asarray(pos, np.float32)
    batch = np.asarray(batch, np.int32)
    agent_h = np.asarray(agent_h, np.float32)
    Wf1 = np.asarray(Wf1, np.float32)
    bf1 = np.asarray(bf1, np.float32)
    Wf2 = np.asarray(Wf2, np.float32)
    bf2 = np.asarray(bf2, np.float32)
    Wg1 = np.asarray(Wg1, np.float32)
    bg1 = np.asarray(bg1, np.float32)
    Wg2 = np.asarray(Wg2, np.float32)
    bg2 = np.asarray(bg2, np.float32)

    nc = _build_program()
    in_maps = _make_in_maps(pos, Wf1, bf1, Wf2, bf2, Wg1, bg1, Wg2, bg2)

    res = run_bass_kernel_spmd(nc, in_maps, list(range(NCORES)))
    outs = res.results

    affordances = np.empty((N, 16), np.float32)
    reconstructed = np.empty((N, 3), np.float32)
    coherence_spatial = np.empty((N,), np.float32)

    seg_aff = np.zeros((B, 16), np.float64)
    seg_err = np.zeros((B,), np.float64)
    counts = np.bincount(batch, minlength=B).astype(np.float64)
    starts = np.searchsorted(batch, np.arange(B + 1))

    for c in range(NCORES):
        o = outs[c]
        aff0 = _unperm_aff(o["aff"])[:NC]
        diff = _unperm_diff(o["diff"])[:NC]
        err = _unperm_err(o["errp"])[:NC]
        lo = c * NC
        affordances[lo:lo + NC] = aff0 + bf2[None, :]
        reconstructed[lo:lo + NC] = pos[lo:lo + NC] - diff
        coherence_spatial[lo:lo + NC] = err

        parts = o["part"].reshape(NW, 288)
        aff_w = parts[:, 0:256].reshape(NW, 16, 16).sum(axis=1)
        err_w = parts[:, 256:288].sum(axis=1)

        for w in range(NW):
            g0 = lo + w * WINDOW
            g1 = min(g0 + WINDOW, lo + NC)
            s_lo = batch[g0]
            s_hi = batch[g1 - 1]
            full = (g1 - g0) == WINDOW
            if full and s_lo == s_hi:
                seg_aff[s_lo] += aff_w[w].astype(np.float64)
                seg_err[s_lo] += float(err_w[w])
            else:
                for s in range(s_lo, s_hi + 1):
                    a = max(g0, starts[s])
                    b_ = min(g1, starts[s + 1])
                    if b_ > a:
                        seg_aff[s] += aff0[a - lo:b_ - lo].sum(
                            axis=0, dtype=np.float64)
                        seg_err[s] += err[a - lo:b_ - lo].sum(
                            dtype=np.float64)

    denom = np.maximum(counts, 1.0)
    coherence_signal = (seg_err / denom).astype(np.float32)[:, None]
    batch_aff = (seg_aff / denom[:, None]).astype(np.float32) + bf2[None, :]

    # tiny GRU + heads on host (B=64)
    Wx = np.asarray(Wx, np.float32)
    Wh = np.asarray(Wh, np.float32)
    bx = np.asarray(bx, np.float32)
    bh = np.asarray(bh, np.float32)
    Wlat = np.asarray(Wlat, np.float32)
    blat = np.asarray(blat, np.float32)
    Wact = np.asarray(Wact, np.float32)
    bact = np.asarray(bact, np.float32)

    gx = batch_aff @ Wx + bx
    gh = agent_h @ Wh + bh
    AH = agent_h.shape[1]
    gx_r, gx_z, gx_n = gx[:, :AH], gx[:, AH:2 * AH], gx[:, 2 * AH:]
    gh_r, gh_z, gh_n = gh[:, :AH], gh[:, AH:2 * AH], gh[:, 2 * AH:]

    def sigmoid(v):
        return 1.0 / (1.0 + np.exp(-v))

    r = sigmoid(gx_r + gh_r)
    z = sigmoid(gx_z + gh_z)
    n_ = np.tanh(gx_n + r * gh_n)
    agent_h_next = (1.0 - z) * n_ + z * agent_h
    latent = np.tanh(agent_h_next @ Wlat + blat)
    agent_action = latent @ Wact + bact

    return (affordances, reconstructed, coherence_signal.astype(np.float32),
            coherence_spatial, agent_action.astype(np.float32),
            agent_h_next.astype(np.float32))


# revision 9
# speedup vs baseline: 1.2210x; 1.1153x over previous
"""Trainium2 Bass kernel for nn_AdjunctionModel (segment_reduce).

Math (per point, N=1e6 points, B=64 sorted segments):
    h1   = relu(pos @ Wf1 + bf1)            (N,128)
    aff  = h1 @ Wf2 + bf2                   (N,16)   [output]
    h2   = relu(aff @ Wg1 + bg1)            (N,128)
    rec  = h2 @ Wg2 + bg2                   (N,3)    [output]
    err  = sum((pos - rec)^2, -1)           (N,)     [output]
    per-segment means of err and aff feed a tiny GRU (B=64).

Key algebraic fold: there is no nonlinearity between the two middle
matmuls, so  h2 = relu(h1 @ (Wf2 @ Wg1) + (bf2 @ Wg1 + bg1)).  The
device computes, per 512-point block:
    L1   : h1 = Wf1^T @ posT            (fp32r matmul, N=512)
    relu1: s1 = relu(h1 + bf1)          (ACT, fp16 out)
    L3   : pre2 = W23^T @ s1            (fp16 matmul)
    relu2: s2 = relu(pre2 + b23)        (ACT or DVE, alternating)
    aff0 : Wf2p^T @ s1  -> quadrant 32q of a psum tile   (fp16)
    rec0 : Wg2p^T @ s2  -> quadrant 32q of a second tile (fp16)
Per triad (3 blocks), a DVE 32x32 StreamTranspose turns the quadrant-
packed (96,512) psum tiles into per-point-layout and the per-window
(2 triads) partial sums are reduced by a ones-matmul.  Host combines
per-window partials into per-segment sums (recomputing the few windows
that straddle a segment boundary from the per-point outputs), adds the
deferred biases, and runs the tiny GRU in numpy.

Sharding: data-parallel over points, 8 cores, same NEFF on every core
(per-core inputs differ only in data).
"""

import os
import sys
from contextlib import ExitStack

import numpy as np

sys.path.insert(0, "/opt/trn_rl_repo")

import ml_dtypes  # noqa: E402
import concourse.bass as bass  # noqa: E402
import concourse.tile as tile  # noqa: E402
from concourse import bacc, mybir  # noqa: E402
from concourse.bass_utils import run_bass_kernel_spmd  # noqa: E402

F32 = mybir.dt.float32
F32R = mybir.dt.float32r
F16 = mybir.dt.float16
BF16 = mybir.dt.bfloat16
HID_DT = BF16 if os.environ.get("KHID", "f16") == "bf16" else F16
AF = mybir.ActivationFunctionType
ALU = mybir.AluOpType
AX = mybir.AxisListType

N = 1_000_000
B = 64
NCORES = 8
NC = N // NCORES           # 125000 points per core
BLOCK = 512
TRIAD = 3 * BLOCK          # 1536
WINDOW = 2 * TRIAD         # 3072
NW = (NC + WINDOW - 1) // WINDOW   # 41
NPAD = NW * WINDOW         # 125952
NTRIAD = 2 * NW            # 82

# relu2 engine assignment: ACT on even blocks, DVE on odd (≈0.5 split)
def _relu2_on_act(blk: int) -> bool:
    return blk % 2 == 0


_CACHE = {}


def _build_program():
    if "prog" in _CACHE:
        return _CACHE["prog"]

    nc = bacc.Bacc("TRN2", target_bir_lowering=False, debug=False,
                   num_devices=NCORES)

    # ---- DRAM I/O ----
    d_posT = nc.dram_tensor("posT", [3, NPAD], F32, kind="ExternalInput").ap()
    d_posb = nc.dram_tensor("posb", [NTRIAD, 96, 48], F32,
                            kind="ExternalInput").ap()
    d_Wf1 = nc.dram_tensor("Wf1", [3, 128], F32, kind="ExternalInput").ap()
    d_W23 = nc.dram_tensor("W23", [128, 128], HID_DT, kind="ExternalInput").ap()
    d_Wf2p = nc.dram_tensor("Wf2p", [128, 32], HID_DT, kind="ExternalInput").ap()
    d_Wg2p = nc.dram_tensor("Wg2p", [128, 32], HID_DT, kind="ExternalInput").ap()
    d_bf1 = nc.dram_tensor("bf1", [128, 1], F32, kind="ExternalInput").ap()
    d_b23 = nc.dram_tensor("b23", [128, 1], F32, kind="ExternalInput").ap()

    d_aff = nc.dram_tensor("aff", [NTRIAD, 96, 16, 16], F32,
                           kind="ExternalOutput").ap()
    d_diff = nc.dram_tensor("diff", [NTRIAD, 96, 48], F32,
                            kind="ExternalOutput").ap()
    d_errp = nc.dram_tensor("errp", [NW, 96, 32], F32,
                            kind="ExternalOutput").ap()
    d_part = nc.dram_tensor("part", [1, NW * 288], F32,
                            kind="ExternalOutput").ap()

    with tile.TileContext(nc) as tc, ExitStack() as ctx:
        consts = ctx.enter_context(tc.tile_pool(name="consts", bufs=1))
        pposT = ctx.enter_context(tc.tile_pool(name="pposT", bufs=3))
        pposb = ctx.enter_context(tc.tile_pool(name="pposb", bufs=3))
        ps1 = ctx.enter_context(tc.tile_pool(name="ps1", bufs=7))
        ps2 = ctx.enter_context(tc.tile_pool(name="ps2", bufs=7))
        pT = ctx.enter_context(tc.tile_pool(name="pT", bufs=2))
        pdiff = ctx.enter_context(tc.tile_pool(name="pdiff", bufs=2))
        pacc = ctx.enter_context(tc.tile_pool(name="pacc", bufs=3))
        ppart = ctx.enter_context(tc.tile_pool(name="ppart", bufs=1))
        psAh = ctx.enter_context(tc.tile_pool(name="psAh", bufs=3,
                                              space="PSUM"))
        psP = ctx.enter_context(tc.tile_pool(name="psP", bufs=3,
                                             space="PSUM"))
        psQ = ctx.enter_context(tc.tile_pool(name="psQ", bufs=1,
                                             space="PSUM"))
        psR = ctx.enter_context(tc.tile_pool(name="psR", bufs=1,
                                             space="PSUM"))

        Wf1 = consts.tile([3, 128], F32R)
        nc.sync.dma_start(out=Wf1, in_=d_Wf1.bitcast(F32R))
        W23 = consts.tile([128, 128], HID_DT)
        nc.sync.dma_start(out=W23, in_=d_W23)
        Wf2p = consts.tile([128, 32], HID_DT)
        nc.sync.dma_start(out=Wf2p, in_=d_Wf2p)
        Wg2p = consts.tile([128, 32], HID_DT)
        nc.sync.dma_start(out=Wg2p, in_=d_Wg2p)
        bf1 = consts.tile([128, 1], F32)
        nc.sync.dma_start(out=bf1, in_=d_bf1)
        b23 = consts.tile([128, 1], F32)
        nc.sync.dma_start(out=b23, in_=d_b23)
        ones = consts.tile([96, 1], F32)
        nc.vector.memset(ones[:], 1.0)

        partials = ppart.tile([1, NW * 288], F32)

        # Phase-batched software pipeline over triad-slots k:
        #   [L1 x3 (k)] [L3' x3 (k-1)] [aff x3 (k-1)] [rec x3 (k-2)]
        # Same stationary weights within each phase -> long dense PE
        # bursts (HAM-warm); every matmul input produced >=1 slot earlier.
        tstate = {}         # per-triad: s1 list, s2 list, aff3, rec3
        win_state = {}      # per-window: posT_w, posb_w, acc

        def load_window(w):
            posT_w = pposT.tile([3, WINDOW], F32R, tag="posT")
            nc.sync.dma_start(
                out=posT_w,
                in_=d_posT[:, WINDOW * w:WINDOW * (w + 1)].bitcast(F32R))
            posb_w = pposb.tile([96, 96], F32, tag="posb")
            for t in range(2):
                nc.sync.dma_start(out=posb_w[:, 48 * t:48 * (t + 1)],
                                  in_=d_posb[2 * w + t])
            acc = pacc.tile([96, 288], F32, tag="acc")
            win_state[w] = (posT_w, posb_w, acc)

        def front(k):
            # L1 x3 + relu1 x3 for triad k
            w, t = k // 2, k % 2
            for ww in (w, w + 1):
                if ww < NW and ww not in win_state:
                    load_window(ww)
            posT_w = win_state[w][0]
            h1s = []
            for q in range(3):
                cs = BLOCK * (3 * t + q)
                h1 = psAh.tile([128, BLOCK], F32, tag="h1")
                nc.tensor.matmul(h1[:], Wf1[:], posT_w[:, cs:cs + BLOCK],
                                 start=True, stop=True)
                h1s.append(h1)
            s1s = []
            for q in range(3):
                s1 = ps1.tile([128, BLOCK], HID_DT, tag="s1")
                nc.scalar.activation(s1[:], h1s[q][:], AF.Relu, bias=bf1[:])
                s1s.append(s1)
            tstate[k] = {"s1": s1s}

        def mid(k):
            # L3' x3 (+relu2) then aff x3, T_aff, aff DMA, acc for triad k
            w, t = k // 2, k % 2
            st = tstate[k]
            pre2s = []
            for q in range(3):
                pre2 = psP.tile([128, BLOCK], F32, tag="pre2")
                nc.tensor.matmul(pre2[:], W23[:], st["s1"][q][:],
                                 start=True, stop=True)
                pre2s.append(pre2)
            s2s = []
            for q in range(3):
                s2 = ps2.tile([128, BLOCK], HID_DT, tag="s2")
                if _relu2_on_act(3 * k + q):
                    nc.scalar.activation(s2[:], pre2s[q][:], AF.Relu,
                                         bias=b23[:])
                else:
                    nc.vector.tensor_scalar(s2[:], pre2s[q][:], b23[:], 0.0,
                                            ALU.add, ALU.max)
                s2s.append(s2)
            st["s2"] = s2s
            aff3 = psQ.tile([96, BLOCK], F32, name="aff3", tag="affq")
            for q in range(3):
                nc.tensor.matmul(aff3[32 * q:32 * q + 32, :],
                                 Wf2p[:], st["s1"][q][:],
                                 start=True, stop=True)
            st["s1"] = None
            acc = win_state[w][2]
            T_aff = pT.tile([96, BLOCK], F32, tag="Taff")
            nc.vector.transpose(T_aff[:], aff3[:])
            aff_src = T_aff[:, :].rearrange("p (j a) -> p j a",
                                            a=32)[:, :, 0:16]
            nc.sync.dma_start(out=d_aff[k], in_=aff_src)
            if t == 0:
                nc.vector.tensor_copy(acc[:, 0:256], aff_src)
            else:
                nc.vector.tensor_tensor(acc[:, 0:256], acc[:, 0:256],
                                        aff_src, ALU.add)

        def recphase(k):
            # rec x3, T_rec, diff, sq, err-reduce for triad k; flush on odd
            w, t = k // 2, k % 2
            st = tstate[k]
            rec3 = psR.tile([96, BLOCK], F32, name="rec3", tag="rec3")
            for q in range(3):
                nc.tensor.matmul(rec3[32 * q:32 * q + 32, :],
                                 Wg2p[:], st["s2"][q][:],
                                 start=True, stop=True)
            del tstate[k]
            posb_w, acc = win_state[w][1], win_state[w][2]
            T_rec = pT.tile([96, BLOCK], F32, tag="Trec")
            nc.vector.transpose(T_rec[:], rec3[:])
            rec_src = T_rec[:, :].rearrange("p (j c) -> p j c",
                                            c=32)[:, :, 0:3]
            diff = pdiff.tile([96, 48], F32, tag="diff")
            nc.vector.tensor_tensor(diff[:], posb_w[:, 48 * t:48 * (t + 1)],
                                    rec_src, ALU.subtract)
            nc.sync.dma_start(out=d_diff[k], in_=diff[:])
            sq = pdiff.tile([96, 48], F32, tag="sq")
            nc.scalar.activation(sq[:], diff[:], AF.Square)
            nc.vector.reduce_sum(
                acc[:, 256 + 16 * t:256 + 16 * (t + 1)],
                sq[:, :].rearrange("p (j c) -> p j c", c=3),
                axis=AX.X)
            if t == 1:
                fl = psQ.tile([1, 288], F32, name="flush", tag="affq")
                nc.tensor.matmul(fl[0:1, :], ones[:], acc[:],
                                 start=True, stop=True)
                nc.scalar.activation(partials[0:1, 288 * w:288 * (w + 1)],
                                     fl[0:1, :], AF.Copy)
                nc.sync.dma_start(out=d_errp[w], in_=acc[:, 256:288])
                del win_state[w]

        for k in range(NTRIAD + 2):
            if k < NTRIAD:
                front(k)
            if 1 <= k <= NTRIAD:
                mid(k - 1)
            if 2 <= k <= NTRIAD + 1:
                recphase(k - 2)

        nc.sync.dma_start(out=d_part, in_=partials[:])

    nc.compile()
    _CACHE["prog"] = nc
    return nc


def _host_prep(pos, bg2):
    """Per-core input arrays from the full pos."""
    in_maps = []
    for c in range(NCORES):
        chunk = np.zeros((NPAD, 3), np.float32)
        chunk[:NC] = pos[c * NC:(c + 1) * NC]
        posT = np.ascontiguousarray(chunk.T)
        posb = (chunk - bg2[None, :]).reshape(NTRIAD, 3, 16, 32, 3)
        posb = np.ascontiguousarray(posb.transpose(0, 1, 3, 2, 4)
                                    ).reshape(NTRIAD, 96, 48)
        in_maps.append({"posT": posT, "posb": posb})
    return in_maps


def _unperm_aff(aff_perm):
    # (NTRIAD, 96, 16, 16) [t, (q p), j, a] -> (NPAD, 16)
    a = aff_perm.reshape(NTRIAD, 3, 32, 16, 16).transpose(0, 1, 3, 2, 4)
    return np.ascontiguousarray(a).reshape(NPAD, 16)


def _unperm_diff(diff_perm):
    # (NTRIAD, 96, 48) [t, (q p), (j c)] -> (NPAD, 3)
    d = diff_perm.reshape(NTRIAD, 3, 32, 16, 3).transpose(0, 1, 3, 2, 4)
    return np.ascontiguousarray(d).reshape(NPAD, 3)


def _unperm_err(errp):
    # (NW, 96, 32) [w, (q p), (t j)] -> (NPAD,)
    e = errp.reshape(NW, 3, 32, 2, 16).transpose(0, 3, 1, 4, 2)
    return np.ascontiguousarray(e).reshape(NPAD)




def _make_in_maps(pos, Wf1, bf1, Wf2, bf2, Wg1, bg1, Wg2, bg2):
    # folded middle matmul + deferred biases
    W23 = (Wf2.astype(np.float64) @ Wg1.astype(np.float64)).astype(np.float32)
    b23 = (bf2.astype(np.float64) @ Wg1.astype(np.float64)
           + bg1.astype(np.float64)).astype(np.float32)

    np_hid = ml_dtypes.bfloat16 if os.environ.get("KHID", "f16") == "bf16" \
        else np.float16
    Wf2p = np.zeros((128, 32), np_hid)
    Wf2p[:, 0:16] = Wf2.astype(np_hid)
    Wg2p = np.zeros((128, 32), np_hid)
    Wg2p[:, 0:3] = Wg2.astype(np_hid)

    common = {
        "Wf1": np.ascontiguousarray(Wf1),
        "W23": W23.astype(np_hid),
        "Wf2p": Wf2p,
        "Wg2p": Wg2p,
        "bf1": np.ascontiguousarray(bf1.reshape(128, 1)),
        "b23": np.ascontiguousarray(b23.reshape(128, 1)),
    }
    in_maps = _host_prep(pos, bg2)
    for m in in_maps:
        m.update(common)
    return in_maps




def kernel(pos, batch, agent_h, coherence_signal_prev, coherence_spatial_prev,
           Wf1, bf1, Wf2, bf2, Wg1, bg1, Wg2, bg2,
           Wx, Wh, bx, bh, Wlat, blat, Wact, bact):
    pos = np.asarray(pos, np.float32)
    batch = np.asarray(batch, np.int32)
    agent_h = np.asarray(agent_h, np.float32)
    Wf1 = np.asarray(Wf1, np.float32)
    bf1 = np.asarray(bf1, np.float32)
    Wf2 = np.asarray(Wf2, np.float32)
    bf2 = np.asarray(bf2, np.float32)
    Wg1 = np.asarray(Wg1, np.float32)
    bg1 = np.asarray(bg1, np.float32)
    Wg2 = np.asarray(Wg2, np.float32)
    bg2 = np.asarray(bg2, np.float32)

    nc = _build_program()
    in_maps = _make_in_maps(pos, Wf1, bf1, Wf2, bf2, Wg1, bg1, Wg2, bg2)

    res = run_bass_kernel_spmd(nc, in_maps, list(range(NCORES)))
    outs = res.results

    affordances = np.empty((N, 16), np.float32)
    reconstructed = np.empty((N, 3), np.float32)
    coherence_spatial = np.empty((N,), np.float32)

    seg_aff = np.zeros((B, 16), np.float64)
    seg_err = np.zeros((B,), np.float64)
    counts = np.bincount(batch, minlength=B).astype(np.float64)
    starts = np.searchsorted(batch, np.arange(B + 1))

    for c in range(NCORES):
        o = outs[c]
        aff0 = _unperm_aff(o["aff"])[:NC]
        diff = _unperm_diff(o["diff"])[:NC]
        err = _unperm_err(o["errp"])[:NC]
        lo = c * NC
        affordances[lo:lo + NC] = aff0 + bf2[None, :]
        reconstructed[lo:lo + NC] = pos[lo:lo + NC] - diff
        coherence_spatial[lo:lo + NC] = err

        parts = o["part"].reshape(NW, 288)
        aff_w = parts[:, 0:256].reshape(NW, 16, 16).sum(axis=1)
        err_w = parts[:, 256:288].sum(axis=1)

        for w in range(NW):
            g0 = lo + w * WINDOW
            g1 = min(g0 + WINDOW, lo + NC)
            s_lo = batch[g0]
            s_hi = batch[g1 - 1]
            full = (g1 - g0) == WINDOW
            if full and s_lo == s_hi:
                seg_aff[s_lo] += aff_w[w].astype(np.float64)
                seg_err[s_lo] += float(err_w[w])
            else:
                for s in range(s_lo, s_hi + 1):
                    a = max(g0, starts[s])
                    b_ = min(g1, starts[s + 1])
                    if b_ > a:
                        seg_aff[s] += aff0[a - lo:b_ - lo].sum(
                            axis=0, dtype=np.float64)
                        seg_err[s] += err[a - lo:b_ - lo].sum(
                            dtype=np.float64)

    denom = np.maximum(counts, 1.0)
    coherence_signal = (seg_err / denom).astype(np.float32)[:, None]
    batch_aff = (seg_aff / denom[:, None]).astype(np.float32) + bf2[None, :]

    # tiny GRU + heads on host (B=64)
    Wx = np.asarray(Wx, np.float32)
    Wh = np.asarray(Wh, np.float32)
    bx = np.asarray(bx, np.float32)
    bh = np.asarray(bh, np.float32)
    Wlat = np.asarray(Wlat, np.float32)
    blat = np.asarray(blat, np.float32)
    Wact = np.asarray(Wact, np.float32)
    bact = np.asarray(bact, np.float32)

    gx = batch_aff @ Wx + bx
    gh = agent_h @ Wh + bh
    AH = agent_h.shape[1]
    gx_r, gx_z, gx_n = gx[:, :AH], gx[:, AH:2 * AH], gx[:, 2 * AH:]
    gh_r, gh_z, gh_n = gh[:, :AH], gh[:, AH:2 * AH], gh[:, 2 * AH:]

    def sigmoid(v):
        return 1.0 / (1.0 + np.exp(-v))

    r = sigmoid(gx_r + gh_r)
    z = sigmoid(gx_z + gh_z)
    n_ = np.tanh(gx_n + r * gh_n)
    agent_h_next = (1.0 - z) * n_ + z * agent_h
    latent = np.tanh(agent_h_next @ Wlat + blat)
    agent_action = latent @ Wact + bact

    return (affordances, reconstructed, coherence_signal.astype(np.float32),
            coherence_spatial, agent_action.astype(np.float32),
            agent_h_next.astype(np.float32))


# revision 16
# speedup vs baseline: 1.4150x; 1.1589x over previous
"""Trainium2 Bass kernel for nn_AdjunctionModel (segment_reduce).

Math (per point, N=1e6 points, B=64 sorted segments):
    h1   = relu(pos @ Wf1 + bf1)            (N,128)
    aff  = h1 @ Wf2 + bf2                   (N,16)   [output]
    h2   = relu(aff @ Wg1 + bg1)            (N,128)
    rec  = h2 @ Wg2 + bg2                   (N,3)    [output]
    err  = sum((pos - rec)^2, -1)           (N,)     [output]
    per-segment means of err and aff feed a tiny GRU (B=64).

Key algebraic fold: there is no nonlinearity between the two middle
matmuls, so  h2 = relu(h1 @ (Wf2 @ Wg1) + (bf2 @ Wg1 + bg1)).  The
device computes, per 512-point block:
    L1   : h1 = Wf1^T @ posT            (fp32r matmul, N=512)
    relu1: s1 = relu(h1 + bf1)          (ACT, fp16 out)
    L3   : pre2 = W23^T @ s1            (fp16 matmul)
    relu2: s2 = relu(pre2 + b23)        (ACT or DVE, alternating)
    aff0 : Wf2p^T @ s1  -> quadrant 32q of a psum tile   (fp16)
    rec0 : Wg2p^T @ s2  -> quadrant 32q of a second tile (fp16)
Per triad (3 blocks), a DVE 32x32 StreamTranspose turns the quadrant-
packed (96,512) psum tiles into per-point-layout and the per-window
(2 triads) partial sums are reduced by a ones-matmul.  Host combines
per-window partials into per-segment sums (recomputing the few windows
that straddle a segment boundary from the per-point outputs), adds the
deferred biases, and runs the tiny GRU in numpy.

Sharding: data-parallel over points, 8 cores, same NEFF on every core
(per-core inputs differ only in data).
"""

import os
import sys
from contextlib import ExitStack

import numpy as np

sys.path.insert(0, "/opt/trn_rl_repo")

import ml_dtypes  # noqa: E402
import concourse.bass as bass  # noqa: E402
import concourse.tile as tile  # noqa: E402
from concourse.tile import add_dep_helper  # noqa: E402
from concourse import bacc, mybir  # noqa: E402
from concourse.bass_utils import run_bass_kernel_spmd  # noqa: E402

F32 = mybir.dt.float32
F32R = mybir.dt.float32r
F16 = mybir.dt.float16
BF16 = mybir.dt.bfloat16
HID_DT = BF16 if os.environ.get("KHID", "f16") == "bf16" else F16
AF = mybir.ActivationFunctionType
ALU = mybir.AluOpType
AX = mybir.AxisListType

N = 1_000_000
B = 64
NCORES = 8
NC = N // NCORES           # 125000 points per core
BLOCK = 512
TRIAD = 3 * BLOCK          # 1536
WINDOW = 2 * TRIAD         # 3072
NW = (NC + WINDOW - 1) // WINDOW   # 41
NPAD = NW * WINDOW         # 125952
NTRIAD = 2 * NW            # 82

# relu2 engine assignment: ACT on even blocks, DVE on odd (≈0.5 split)
def _relu2_on_act(blk: int) -> bool:
    return blk % 2 == 0


_CACHE = {}


def _build_program():
    if "prog" in _CACHE:
        return _CACHE["prog"]

    nc = bacc.Bacc("TRN2", target_bir_lowering=False, debug=False,
                   num_devices=NCORES)

    # ---- DRAM I/O ----
    d_posT = nc.dram_tensor("posT", [3, NPAD], F32, kind="ExternalInput").ap()
    d_posb = nc.dram_tensor("posb", [NTRIAD, 96, 48], F32,
                            kind="ExternalInput").ap()
    d_Wf1 = nc.dram_tensor("Wf1", [3, 128], F32, kind="ExternalInput").ap()
    d_W23 = nc.dram_tensor("W23", [128, 128], HID_DT, kind="ExternalInput").ap()
    d_Wf2p = nc.dram_tensor("Wf2p", [128, 32], HID_DT, kind="ExternalInput").ap()
    d_Wg2p = nc.dram_tensor("Wg2p", [128, 32], HID_DT, kind="ExternalInput").ap()
    d_bf1 = nc.dram_tensor("bf1", [128, 1], F32, kind="ExternalInput").ap()
    d_b23 = nc.dram_tensor("b23", [128, 1], F32, kind="ExternalInput").ap()

    d_aff = nc.dram_tensor("aff", [NTRIAD, 96, 16, 16], F32,
                           kind="ExternalOutput").ap()
    d_diff = nc.dram_tensor("diff", [NTRIAD, 96, 48], F32,
                            kind="ExternalOutput").ap()
    d_errp = nc.dram_tensor("errp", [NW, 96, 32], F32,
                            kind="ExternalOutput").ap()
    d_part = nc.dram_tensor("part", [1, NW * 288], F32,
                            kind="ExternalOutput").ap()

    with tile.TileContext(nc) as tc, ExitStack() as ctx:
        consts = ctx.enter_context(tc.tile_pool(name="consts", bufs=1))
        pposT = ctx.enter_context(tc.tile_pool(name="pposT", bufs=5))
        pposb = ctx.enter_context(tc.tile_pool(name="pposb", bufs=5))
        ps1 = ctx.enter_context(tc.tile_pool(name="ps1", bufs=7))
        ps2 = ctx.enter_context(tc.tile_pool(name="ps2", bufs=7))
        pT = ctx.enter_context(tc.tile_pool(name="pT", bufs=2))
        pdiff = ctx.enter_context(tc.tile_pool(name="pdiff", bufs=2))
        pacc = ctx.enter_context(tc.tile_pool(name="pacc", bufs=5))
        ppart = ctx.enter_context(tc.tile_pool(name="ppart", bufs=1))
        psAh = ctx.enter_context(tc.tile_pool(name="psAh", bufs=2,
                                              space="PSUM"))
        psF = ctx.enter_context(tc.tile_pool(name="psF", bufs=1,
                                             space="PSUM"))
        psP = ctx.enter_context(tc.tile_pool(name="psP", bufs=3,
                                             space="PSUM"))
        psQ = ctx.enter_context(tc.tile_pool(name="psQ", bufs=1,
                                             space="PSUM"))
        psR = ctx.enter_context(tc.tile_pool(name="psR", bufs=1,
                                             space="PSUM"))

        Wf1 = consts.tile([3, 128], F32R)
        nc.sync.dma_start(out=Wf1, in_=d_Wf1.bitcast(F32R))
        W23 = consts.tile([128, 128], HID_DT)
        nc.sync.dma_start(out=W23, in_=d_W23)
        Wf2p = consts.tile([128, 32], HID_DT)
        nc.sync.dma_start(out=Wf2p, in_=d_Wf2p)
        Wg2p = consts.tile([128, 32], HID_DT)
        nc.sync.dma_start(out=Wg2p, in_=d_Wg2p)
        bf1 = consts.tile([128, 1], F32)
        nc.sync.dma_start(out=bf1, in_=d_bf1)
        b23 = consts.tile([128, 1], F32)
        nc.sync.dma_start(out=b23, in_=d_b23)
        ones = consts.tile([96, 1], F32)
        nc.vector.memset(ones[:], 1.0)

        partials = ppart.tile([1, NW * 288], F32)

        _pe_prev = [None]

        def mm(out, lhsT, rhs):
            r = nc.tensor.matmul(out, lhsT, rhs, start=True, stop=True)
            if _pe_prev[0] is not None and os.environ.get("KCHAIN", "1") == "1":
                add_dep_helper(r.ins, _pe_prev[0],
                               sync=os.environ.get("KSYNC", "0") == "1",
                               reason="pe stream order")
            _pe_prev[0] = r.ins
            return r

        def phase_break():
            if os.environ.get("KPBREAK", "0") == "1":
                _pe_prev[0] = None

        # Phase-batched software pipeline over triad-slots k:
        #   [L1 x3 (k)] [L3' x3 (k-1)] [aff x3 (k-1)] [rec x3 (k-2)]
        # Same stationary weights within each phase -> long dense PE
        # bursts (HAM-warm); every matmul input produced >=1 slot earlier.
        tstate = {}         # per-triad: s1 list, s2 list, aff3, rec3
        win_state = {}      # per-window: posT_w, posb_w, acc

        def load_window(w):
            posT_w = pposT.tile([3, WINDOW], F32R, tag="posT")
            nc.sync.dma_start(
                out=posT_w,
                in_=d_posT[:, WINDOW * w:WINDOW * (w + 1)].bitcast(F32R))
            posb_w = pposb.tile([96, 96], F32, tag="posb")
            for t in range(2):
                nc.sync.dma_start(out=posb_w[:, 48 * t:48 * (t + 1)],
                                  in_=d_posb[2 * w + t])
            acc = pacc.tile([96, 288], F32, tag="acc")
            win_state[w] = (posT_w, posb_w, acc)

        def front(k):
            # L1 x3 + relu1 x3 for triad k
            w, t = k // 2, k % 2
            for ww in (w, w + 1):
                if ww < NW and ww not in win_state:
                    load_window(ww)
            posT_w = win_state[w][0]
            phase_break()
            h1s = []
            for q in range(3):
                cs = BLOCK * (3 * t + q)
                h1 = psAh.tile([128, BLOCK], F32, tag="h1")
                mm(h1[:], Wf1[:], posT_w[:, cs:cs + BLOCK])
                h1s.append(h1)
            s1s = []
            for q in range(3):
                s1 = ps1.tile([128, BLOCK], HID_DT, tag="s1")
                nc.scalar.activation(s1[:], h1s[q][:], AF.Relu, bias=bf1[:])
                s1s.append(s1)
            tstate[k] = {"s1": s1s}

        def mid(k):
            # L3' x3 (+relu2) then aff x3, T_aff, aff DMA, acc for triad k
            w, t = k // 2, k % 2
            st = tstate[k]
            phase_break()
            pre2s = []
            for q in range(3):
                pre2 = psP.tile([128, BLOCK], F32, tag="pre2")
                mm(pre2[:], W23[:], st["s1"][q][:])
                pre2s.append(pre2)
            s2s = []
            for q in range(3):
                s2 = ps2.tile([128, BLOCK], HID_DT, tag="s2")
                if _relu2_on_act(3 * k + q):
                    nc.scalar.activation(s2[:], pre2s[q][:], AF.Relu,
                                         bias=b23[:])
                else:
                    nc.vector.tensor_scalar(s2[:], pre2s[q][:], b23[:], 0.0,
                                            ALU.add, ALU.max)
                s2s.append(s2)
            st["s2"] = s2s
            phase_break()
            aff3 = psQ.tile([96, BLOCK], F32, name="aff3", tag="affq")
            for q in range(3):
                mm(aff3[32 * q:32 * q + 32, :], Wf2p[:], st["s1"][q][:])
            st["s1"] = None
            acc = win_state[w][2]
            T_aff = pT.tile([96, BLOCK], F32, tag="Taff")
            nc.vector.transpose(T_aff[:], aff3[:])
            aff_src = T_aff[:, :].rearrange("p (j a) -> p j a",
                                            a=32)[:, :, 0:16]
            nc.sync.dma_start(out=d_aff[k], in_=aff_src)
            if t == 0:
                nc.vector.tensor_copy(acc[:, 0:256], aff_src)
            else:
                nc.vector.tensor_tensor(acc[:, 0:256], acc[:, 0:256],
                                        aff_src, ALU.add)

        def do_flush(w):
            phase_break()
            acc = win_state[w][2]
            fl = psF.tile([1, 288], F32, name="flush", tag="flush")
            mm(fl[0:1, :], ones[:], acc[:])
            nc.scalar.activation(partials[0:1, 288 * w:288 * (w + 1)],
                                 fl[0:1, :], AF.Copy)
            nc.sync.dma_start(out=d_errp[w], in_=acc[:, 256:288])
            del win_state[w]

        def recphase(k):
            # rec x3, T_rec, diff, sq, err-reduce for triad k
            w, t = k // 2, k % 2
            if k >= 1 and k % 2 == 0:
                do_flush(k // 2 - 1)
            st = tstate[k]
            phase_break()
            rec3 = psR.tile([96, BLOCK], F32, name="rec3", tag="rec3")
            for q in range(3):
                mm(rec3[32 * q:32 * q + 32, :], Wg2p[:], st["s2"][q][:])
            del tstate[k]
            posb_w, acc = win_state[w][1], win_state[w][2]
            T_rec = pT.tile([96, BLOCK], F32, tag="Trec")
            nc.vector.transpose(T_rec[:], rec3[:])
            rec_src = T_rec[:, :].rearrange("p (j c) -> p j c",
                                            c=32)[:, :, 0:3]
            diff = pdiff.tile([96, 48], F32, tag="diff")
            nc.vector.tensor_tensor(diff[:], posb_w[:, 48 * t:48 * (t + 1)],
                                    rec_src, ALU.subtract)
            nc.sync.dma_start(out=d_diff[k], in_=diff[:])
            sq = pdiff.tile([96, 48], F32, tag="sq")
            nc.scalar.activation(sq[:], diff[:], AF.Square)
            nc.vector.reduce_sum(
                acc[:, 256 + 16 * t:256 + 16 * (t + 1)],
                sq[:, :].rearrange("p (j c) -> p j c", c=3),
                axis=AX.X)

        for k in range(NTRIAD + 2):
            if k < NTRIAD:
                front(k)
            if 1 <= k <= NTRIAD:
                mid(k - 1)
            if 2 <= k <= NTRIAD + 1:
                recphase(k - 2)
        do_flush(NW - 1)

        nc.sync.dma_start(out=d_part, in_=partials[:])

    nc.compile()
    _CACHE["prog"] = nc
    return nc


def _host_prep(pos, bg2):
    """Per-core input arrays from the full pos."""
    in_maps = []
    for c in range(NCORES):
        chunk = np.zeros((NPAD, 3), np.float32)
        chunk[:NC] = pos[c * NC:(c + 1) * NC]
        posT = np.ascontiguousarray(chunk.T)
        posb = (chunk - bg2[None, :]).reshape(NTRIAD, 3, 16, 32, 3)
        posb = np.ascontiguousarray(posb.transpose(0, 1, 3, 2, 4)
                                    ).reshape(NTRIAD, 96, 48)
        in_maps.append({"posT": posT, "posb": posb})
    return in_maps


def _unperm_aff(aff_perm):
    # (NTRIAD, 96, 16, 16) [t, (q p), j, a] -> (NPAD, 16)
    a = aff_perm.reshape(NTRIAD, 3, 32, 16, 16).transpose(0, 1, 3, 2, 4)
    return np.ascontiguousarray(a).reshape(NPAD, 16)


def _unperm_diff(diff_perm):
    # (NTRIAD, 96, 48) [t, (q p), (j c)] -> (NPAD, 3)
    d = diff_perm.reshape(NTRIAD, 3, 32, 16, 3).transpose(0, 1, 3, 2, 4)
    return np.ascontiguousarray(d).reshape(NPAD, 3)


def _unperm_err(errp):
    # (NW, 96, 32) [w, (q p), (t j)] -> (NPAD,)
    e = errp.reshape(NW, 3, 32, 2, 16).transpose(0, 3, 1, 4, 2)
    return np.ascontiguousarray(e).reshape(NPAD)




def _make_in_maps(pos, Wf1, bf1, Wf2, bf2, Wg1, bg1, Wg2, bg2):
    # folded middle matmul + deferred biases
    W23 = (Wf2.astype(np.float64) @ Wg1.astype(np.float64)).astype(np.float32)
    b23 = (bf2.astype(np.float64) @ Wg1.astype(np.float64)
           + bg1.astype(np.float64)).astype(np.float32)

    np_hid = ml_dtypes.bfloat16 if os.environ.get("KHID", "f16") == "bf16" \
        else np.float16
    Wf2p = np.zeros((128, 32), np_hid)
    Wf2p[:, 0:16] = Wf2.astype(np_hid)
    Wg2p = np.zeros((128, 32), np_hid)
    Wg2p[:, 0:3] = Wg2.astype(np_hid)

    common = {
        "Wf1": np.ascontiguousarray(Wf1),
        "W23": W23.astype(np_hid),
        "Wf2p": Wf2p,
        "Wg2p": Wg2p,
        "bf1": np.ascontiguousarray(bf1.reshape(128, 1)),
        "b23": np.ascontiguousarray(b23.reshape(128, 1)),
    }
    in_maps = _host_prep(pos, bg2)
    for m in in_maps:
        m.update(common)
    return in_maps




def kernel(pos, batch, agent_h, coherence_signal_prev, coherence_spatial_prev,
           Wf1, bf1, Wf2, bf2, Wg1, bg1, Wg2, bg2,
           Wx, Wh, bx, bh, Wlat, blat, Wact, bact):
    pos = np.asarray(pos, np.float32)
    batch = np.asarray(batch, np.int32)
    agent_h = np.asarray(agent_h, np.float32)
    Wf1 = np.asarray(Wf1, np.float32)
    bf1 = np.asarray(bf1, np.float32)
    Wf2 = np.asarray(Wf2, np.float32)
    bf2 = np.asarray(bf2, np.float32)
    Wg1 = np.asarray(Wg1, np.float32)
    bg1 = np.asarray(bg1, np.float32)
    Wg2 = np.asarray(Wg2, np.float32)
    bg2 = np.asarray(bg2, np.float32)

    nc = _build_program()
    in_maps = _make_in_maps(pos, Wf1, bf1, Wf2, bf2, Wg1, bg1, Wg2, bg2)

    res = run_bass_kernel_spmd(nc, in_maps, list(range(NCORES)))
    outs = res.results

    affordances = np.empty((N, 16), np.float32)
    reconstructed = np.empty((N, 3), np.float32)
    coherence_spatial = np.empty((N,), np.float32)

    seg_aff = np.zeros((B, 16), np.float64)
    seg_err = np.zeros((B,), np.float64)
    counts = np.bincount(batch, minlength=B).astype(np.float64)
    starts = np.searchsorted(batch, np.arange(B + 1))

    for c in range(NCORES):
        o = outs[c]
        aff0 = _unperm_aff(o["aff"])[:NC]
        diff = _unperm_diff(o["diff"])[:NC]
        err = _unperm_err(o["errp"])[:NC]
        lo = c * NC
        affordances[lo:lo + NC] = aff0 + bf2[None, :]
        reconstructed[lo:lo + NC] = pos[lo:lo + NC] - diff
        coherence_spatial[lo:lo + NC] = err

        parts = o["part"].reshape(NW, 288)
        aff_w = parts[:, 0:256].reshape(NW, 16, 16).sum(axis=1)
        err_w = parts[:, 256:288].sum(axis=1)

        for w in range(NW):
            g0 = lo + w * WINDOW
            g1 = min(g0 + WINDOW, lo + NC)
            s_lo = batch[g0]
            s_hi = batch[g1 - 1]
            full = (g1 - g0) == WINDOW
            if full and s_lo == s_hi:
                seg_aff[s_lo] += aff_w[w].astype(np.float64)
                seg_err[s_lo] += float(err_w[w])
            else:
                for s in range(s_lo, s_hi + 1):
                    a = max(g0, starts[s])
                    b_ = min(g1, starts[s + 1])
                    if b_ > a:
                        seg_aff[s] += aff0[a - lo:b_ - lo].sum(
                            axis=0, dtype=np.float64)
                        seg_err[s] += err[a - lo:b_ - lo].sum(
                            dtype=np.float64)

    denom = np.maximum(counts, 1.0)
    coherence_signal = (seg_err / denom).astype(np.float32)[:, None]
    batch_aff = (seg_aff / denom[:, None]).astype(np.float32) + bf2[None, :]

    # tiny GRU + heads on host (B=64)
    Wx = np.asarray(Wx, np.float32)
    Wh = np.asarray(Wh, np.float32)
    bx = np.asarray(bx, np.float32)
    bh = np.asarray(bh, np.float32)
    Wlat = np.asarray(Wlat, np.float32)
    blat = np.asarray(blat, np.float32)
    Wact = np.asarray(Wact, np.float32)
    bact = np.asarray(bact, np.float32)

    gx = batch_aff @ Wx + bx
    gh = agent_h @ Wh + bh
    AH = agent_h.shape[1]
    gx_r, gx_z, gx_n = gx[:, :AH], gx[:, AH:2 * AH], gx[:, 2 * AH:]
    gh_r, gh_z, gh_n = gh[:, :AH], gh[:, AH:2 * AH], gh[:, 2 * AH:]

    def sigmoid(v):
        return 1.0 / (1.0 + np.exp(-v))

    r = sigmoid(gx_r + gh_r)
    z = sigmoid(gx_z + gh_z)
    n_ = np.tanh(gx_n + r * gh_n)
    agent_h_next = (1.0 - z) * n_ + z * agent_h
    latent = np.tanh(agent_h_next @ Wlat + blat)
    agent_action = latent @ Wact + bact

    return (affordances, reconstructed, coherence_signal.astype(np.float32),
            coherence_spatial, agent_action.astype(np.float32),
            agent_h_next.astype(np.float32))


# revision 17
# speedup vs baseline: 1.5826x; 1.1184x over previous
"""Trainium2 Bass kernel for nn_AdjunctionModel (segment_reduce).

Math (per point, N=1e6 points, B=64 sorted segments):
    h1   = relu(pos @ Wf1 + bf1)            (N,128)
    aff  = h1 @ Wf2 + bf2                   (N,16)   [output]
    h2   = relu(aff @ Wg1 + bg1)            (N,128)
    rec  = h2 @ Wg2 + bg2                   (N,3)    [output]
    err  = sum((pos - rec)^2, -1)           (N,)     [output]
    per-segment means of err and aff feed a tiny GRU (B=64).

Key algebraic fold: there is no nonlinearity between the two middle
matmuls, so  h2 = relu(h1 @ (Wf2 @ Wg1) + (bf2 @ Wg1 + bg1)).  The
device computes, per 512-point block:
    L1   : h1 = Wf1^T @ posT            (fp32r matmul, N=512)
    relu1: s1 = relu(h1 + bf1)          (ACT, fp16 out)
    L3   : pre2 = W23^T @ s1            (fp16 matmul)
    relu2: s2 = relu(pre2 + b23)        (ACT or DVE, alternating)
    aff0 : Wf2p^T @ s1  -> quadrant 32q of a psum tile   (fp16)
    rec0 : Wg2p^T @ s2  -> quadrant 32q of a second tile (fp16)
Per triad (3 blocks), a DVE 32x32 StreamTranspose turns the quadrant-
packed (96,512) psum tiles into per-point-layout and the per-window
(2 triads) partial sums are reduced by a ones-matmul.  Host combines
per-window partials into per-segment sums (recomputing the few windows
that straddle a segment boundary from the per-point outputs), adds the
deferred biases, and runs the tiny GRU in numpy.

Sharding: data-parallel over points, 8 cores, same NEFF on every core
(per-core inputs differ only in data).
"""

import os
import sys
from contextlib import ExitStack

import numpy as np

sys.path.insert(0, "/opt/trn_rl_repo")

import ml_dtypes  # noqa: E402
import concourse.bass as bass  # noqa: E402
import concourse.tile as tile  # noqa: E402
from concourse.tile import add_dep_helper  # noqa: E402
from concourse import bacc, mybir  # noqa: E402
from concourse.bass_utils import run_bass_kernel_spmd  # noqa: E402

F32 = mybir.dt.float32
F32R = mybir.dt.float32r
F16 = mybir.dt.float16
BF16 = mybir.dt.bfloat16
HID_DT = BF16 if os.environ.get("KHID", "f16") == "bf16" else F16
AF = mybir.ActivationFunctionType
ALU = mybir.AluOpType
AX = mybir.AxisListType

N = 1_000_000
B = 64
NCORES = 8
NC = N // NCORES           # 125000 points per core
BLOCK = 512
TRIAD = 3 * BLOCK          # 1536
WINDOW = 2 * TRIAD         # 3072
NW = (NC + WINDOW - 1) // WINDOW   # 41
NPAD = NW * WINDOW         # 125952
NTRIAD = 2 * NW            # 82

# relu2 engine assignment: ACT on even blocks, DVE on odd (≈0.5 split)
def _relu2_on_act(blk: int) -> bool:
    return blk % 2 == 0


_CACHE = {}


def _build_program():
    if "prog" in _CACHE:
        return _CACHE["prog"]

    nc = bacc.Bacc("TRN2", target_bir_lowering=False, debug=False,
                   num_devices=NCORES)

    # ---- DRAM I/O ----
    d_posT = nc.dram_tensor("posT", [3, NPAD], F16, kind="ExternalInput").ap()
    d_posb = nc.dram_tensor("posb", [NTRIAD, 96, 48], F32,
                            kind="ExternalInput").ap()
    d_Wf1 = nc.dram_tensor("Wf1", [3, 128], F16, kind="ExternalInput").ap()
    d_W23 = nc.dram_tensor("W23", [128, 128], HID_DT, kind="ExternalInput").ap()
    d_Wf2p = nc.dram_tensor("Wf2p", [128, 32], HID_DT, kind="ExternalInput").ap()
    d_Wg2p = nc.dram_tensor("Wg2p", [128, 32], HID_DT, kind="ExternalInput").ap()
    d_bf1 = nc.dram_tensor("bf1", [128, 1], F32, kind="ExternalInput").ap()
    d_b23 = nc.dram_tensor("b23", [128, 1], F32, kind="ExternalInput").ap()

    d_aff = nc.dram_tensor("aff", [NTRIAD, 96, 16, 16], F32,
                           kind="ExternalOutput").ap()
    d_diff = nc.dram_tensor("diff", [NTRIAD, 96, 48], F32,
                            kind="ExternalOutput").ap()
    d_errp = nc.dram_tensor("errp", [NW, 96, 32], F32,
                            kind="ExternalOutput").ap()
    d_part = nc.dram_tensor("part", [1, NW * 288], F32,
                            kind="ExternalOutput").ap()

    with tile.TileContext(nc) as tc, ExitStack() as ctx:
        consts = ctx.enter_context(tc.tile_pool(name="consts", bufs=1))
        pposT = ctx.enter_context(tc.tile_pool(name="pposT", bufs=5))
        pposb = ctx.enter_context(tc.tile_pool(name="pposb", bufs=5))
        ps1 = ctx.enter_context(tc.tile_pool(name="ps1", bufs=7))
        ps2 = ctx.enter_context(tc.tile_pool(name="ps2", bufs=7))
        pT = ctx.enter_context(tc.tile_pool(name="pT", bufs=2))
        pdiff = ctx.enter_context(tc.tile_pool(name="pdiff", bufs=2))
        pacc = ctx.enter_context(tc.tile_pool(name="pacc", bufs=5))
        ppart = ctx.enter_context(tc.tile_pool(name="ppart", bufs=1))
        psAh = ctx.enter_context(tc.tile_pool(name="psAh", bufs=2,
                                              space="PSUM"))
        psF = ctx.enter_context(tc.tile_pool(name="psF", bufs=1,
                                             space="PSUM"))
        psP = ctx.enter_context(tc.tile_pool(name="psP", bufs=3,
                                             space="PSUM"))
        psQ = ctx.enter_context(tc.tile_pool(name="psQ", bufs=1,
                                             space="PSUM"))
        psR = ctx.enter_context(tc.tile_pool(name="psR", bufs=1,
                                             space="PSUM"))

        Wf1 = consts.tile([3, 128], F16)
        nc.sync.dma_start(out=Wf1, in_=d_Wf1)
        W23 = consts.tile([128, 128], HID_DT)
        nc.sync.dma_start(out=W23, in_=d_W23)
        Wf2p = consts.tile([128, 32], HID_DT)
        nc.sync.dma_start(out=Wf2p, in_=d_Wf2p)
        Wg2p = consts.tile([128, 32], HID_DT)
        nc.sync.dma_start(out=Wg2p, in_=d_Wg2p)
        bf1 = consts.tile([128, 1], F32)
        nc.sync.dma_start(out=bf1, in_=d_bf1)
        b23 = consts.tile([128, 1], F32)
        nc.sync.dma_start(out=b23, in_=d_b23)
        ones = consts.tile([96, 1], F32)
        nc.vector.memset(ones[:], 1.0)

        partials = ppart.tile([1, NW * 288], F32)

        _pe_prev = [None]

        def mm(out, lhsT, rhs):
            r = nc.tensor.matmul(out, lhsT, rhs, start=True, stop=True)
            if _pe_prev[0] is not None and os.environ.get("KCHAIN", "1") == "1":
                add_dep_helper(r.ins, _pe_prev[0],
                               sync=os.environ.get("KSYNC", "0") == "1",
                               reason="pe stream order")
            _pe_prev[0] = r.ins
            return r

        def phase_break():
            if os.environ.get("KPBREAK", "0") == "1":
                _pe_prev[0] = None

        # Phase-batched software pipeline over triad-slots k:
        #   [L1 x3 (k)] [L3' x3 (k-1)] [aff x3 (k-1)] [rec x3 (k-2)]
        # Same stationary weights within each phase -> long dense PE
        # bursts (HAM-warm); every matmul input produced >=1 slot earlier.
        tstate = {}         # per-triad: s1 list, s2 list, aff3, rec3
        win_state = {}      # per-window: posT_w, posb_w, acc

        def load_window(w):
            posT_w = pposT.tile([3, WINDOW], F16, tag="posT")
            nc.sync.dma_start(
                out=posT_w,
                in_=d_posT[:, WINDOW * w:WINDOW * (w + 1)])
            posb_w = pposb.tile([96, 96], F32, tag="posb")
            for t in range(2):
                nc.sync.dma_start(out=posb_w[:, 48 * t:48 * (t + 1)],
                                  in_=d_posb[2 * w + t])
            acc = pacc.tile([96, 288], F32, tag="acc")
            win_state[w] = (posT_w, posb_w, acc)

        def front(k):
            # L1 x3 + relu1 x3 for triad k
            w, t = k // 2, k % 2
            for ww in (w, w + 1):
                if ww < NW and ww not in win_state:
                    load_window(ww)
            posT_w = win_state[w][0]
            phase_break()
            h1s = []
            for q in range(3):
                cs = BLOCK * (3 * t + q)
                h1 = psAh.tile([128, BLOCK], F32, tag="h1")
                mm(h1[:], Wf1[:], posT_w[:, cs:cs + BLOCK])
                h1s.append(h1)
            s1s = []
            for q in range(3):
                s1 = ps1.tile([128, BLOCK], HID_DT, tag="s1")
                nc.scalar.activation(s1[:], h1s[q][:], AF.Relu, bias=bf1[:])
                s1s.append(s1)
            tstate[k] = {"s1": s1s}

        def mid(k):
            # L3' x3 (+relu2) then aff x3, T_aff, aff DMA, acc for triad k
            w, t = k // 2, k % 2
            st = tstate[k]
            phase_break()
            pre2s = []
            for q in range(3):
                pre2 = psP.tile([128, BLOCK], F32, tag="pre2")
                mm(pre2[:], W23[:], st["s1"][q][:])
                pre2s.append(pre2)
            s2s = []
            for q in range(3):
                s2 = ps2.tile([128, BLOCK], HID_DT, tag="s2")
                if _relu2_on_act(3 * k + q):
                    nc.scalar.activation(s2[:], pre2s[q][:], AF.Relu,
                                         bias=b23[:])
                else:
                    nc.vector.tensor_scalar(s2[:], pre2s[q][:], b23[:], 0.0,
                                            ALU.add, ALU.max)
                s2s.append(s2)
            st["s2"] = s2s
            phase_break()
            aff3 = psQ.tile([96, BLOCK], F32, name="aff3", tag="affq")
            for q in range(3):
                mm(aff3[32 * q:32 * q + 32, :], Wf2p[:], st["s1"][q][:])
            st["s1"] = None
            acc = win_state[w][2]
            T_aff = pT.tile([96, BLOCK], F32, tag="Taff")
            nc.vector.transpose(T_aff[:], aff3[:])
            aff_src = T_aff[:, :].rearrange("p (j a) -> p j a",
                                            a=32)[:, :, 0:16]
            nc.sync.dma_start(out=d_aff[k], in_=aff_src)
            if t == 0:
                nc.vector.tensor_copy(acc[:, 0:256], aff_src)
            else:
                nc.vector.tensor_tensor(acc[:, 0:256], acc[:, 0:256],
                                        aff_src, ALU.add)

        def do_flush(w):
            phase_break()
            acc = win_state[w][2]
            fl = psF.tile([1, 288], F32, name="flush", tag="flush")
            mm(fl[0:1, :], ones[:], acc[:])
            nc.scalar.activation(partials[0:1, 288 * w:288 * (w + 1)],
                                 fl[0:1, :], AF.Copy)
            nc.sync.dma_start(out=d_errp[w], in_=acc[:, 256:288])
            del win_state[w]

        def recphase(k):
            # rec x3, T_rec, diff, sq, err-reduce for triad k
            w, t = k // 2, k % 2
            if k >= 1 and k % 2 == 0:
                do_flush(k // 2 - 1)
            st = tstate[k]
            phase_break()
            rec3 = psR.tile([96, BLOCK], F32, name="rec3", tag="rec3")
            for q in range(3):
                mm(rec3[32 * q:32 * q + 32, :], Wg2p[:], st["s2"][q][:])
            del tstate[k]
            posb_w, acc = win_state[w][1], win_state[w][2]
            T_rec = pT.tile([96, BLOCK], F32, tag="Trec")
            nc.vector.transpose(T_rec[:], rec3[:])
            rec_src = T_rec[:, :].rearrange("p (j c) -> p j c",
                                            c=32)[:, :, 0:3]
            diff = pdiff.tile([96, 48], F32, tag="diff")
            nc.vector.tensor_tensor(diff[:], posb_w[:, 48 * t:48 * (t + 1)],
                                    rec_src, ALU.subtract)
            nc.sync.dma_start(out=d_diff[k], in_=diff[:])
            sq = pdiff.tile([96, 48], F32, tag="sq")
            nc.scalar.activation(sq[:], diff[:], AF.Square)
            nc.vector.reduce_sum(
                acc[:, 256 + 16 * t:256 + 16 * (t + 1)],
                sq[:, :].rearrange("p (j c) -> p j c", c=3),
                axis=AX.X)

        for k in range(NTRIAD + 2):
            if k < NTRIAD:
                front(k)
            if 1 <= k <= NTRIAD:
                mid(k - 1)
            if 2 <= k <= NTRIAD + 1:
                recphase(k - 2)
        do_flush(NW - 1)

        nc.sync.dma_start(out=d_part, in_=partials[:])

    nc.compile()
    _CACHE["prog"] = nc
    return nc


def _host_prep(pos, bg2):
    """Per-core input arrays from the full pos."""
    in_maps = []
    for c in range(NCORES):
        chunk = np.zeros((NPAD, 3), np.float32)
        chunk[:NC] = pos[c * NC:(c + 1) * NC]
        posT = np.ascontiguousarray(chunk.T.astype(np.float16))
        posb = (chunk - bg2[None, :]).reshape(NTRIAD, 3, 16, 32, 3)
        posb = np.ascontiguousarray(posb.transpose(0, 1, 3, 2, 4)
                                    ).reshape(NTRIAD, 96, 48)
        in_maps.append({"posT": posT, "posb": posb})
    return in_maps


def _unperm_aff(aff_perm):
    # (NTRIAD, 96, 16, 16) [t, (q p), j, a] -> (NPAD, 16)
    a = aff_perm.reshape(NTRIAD, 3, 32, 16, 16).transpose(0, 1, 3, 2, 4)
    return np.ascontiguousarray(a).reshape(NPAD, 16)


def _unperm_diff(diff_perm):
    # (NTRIAD, 96, 48) [t, (q p), (j c)] -> (NPAD, 3)
    d = diff_perm.reshape(NTRIAD, 3, 32, 16, 3).transpose(0, 1, 3, 2, 4)
    return np.ascontiguousarray(d).reshape(NPAD, 3)


def _unperm_err(errp):
    # (NW, 96, 32) [w, (q p), (t j)] -> (NPAD,)
    e = errp.reshape(NW, 3, 32, 2, 16).transpose(0, 3, 1, 4, 2)
    return np.ascontiguousarray(e).reshape(NPAD)




def _make_in_maps(pos, Wf1, bf1, Wf2, bf2, Wg1, bg1, Wg2, bg2):
    # folded middle matmul + deferred biases
    W23 = (Wf2.astype(np.float64) @ Wg1.astype(np.float64)).astype(np.float32)
    b23 = (bf2.astype(np.float64) @ Wg1.astype(np.float64)
           + bg1.astype(np.float64)).astype(np.float32)

    np_hid = ml_dtypes.bfloat16 if os.environ.get("KHID", "f16") == "bf16" \
        else np.float16
    Wf2p = np.zeros((128, 32), np_hid)
    Wf2p[:, 0:16] = Wf2.astype(np_hid)
    Wg2p = np.zeros((128, 32), np_hid)
    Wg2p[:, 0:3] = Wg2.astype(np_hid)

    common = {
        "Wf1": np.ascontiguousarray(Wf1.astype(np.float16)),
        "W23": W23.astype(np_hid),
        "Wf2p": Wf2p,
        "Wg2p": Wg2p,
        "bf1": np.ascontiguousarray(bf1.reshape(128, 1)),
        "b23": np.ascontiguousarray(b23.reshape(128, 1)),
    }
    in_maps = _host_prep(pos, bg2)
    for m in in_maps:
        m.update(common)
    return in_maps




def kernel(pos, batch, agent_h, coherence_signal_prev, coherence_spatial_prev,
           Wf1, bf1, Wf2, bf2, Wg1, bg1, Wg2, bg2,
           Wx, Wh, bx, bh, Wlat, blat, Wact, bact):
    pos = np.asarray(pos, np.float32)
    batch = np.asarray(batch, np.int32)
    agent_h = np.asarray(agent_h, np.float32)
    Wf1 = np.asarray(Wf1, np.float32)
    bf1 = np.asarray(bf1, np.float32)
    Wf2 = np.asarray(Wf2, np.float32)
    bf2 = np.asarray(bf2, np.float32)
    Wg1 = np.asarray(Wg1, np.float32)
    bg1 = np.asarray(bg1, np.float32)
    Wg2 = np.asarray(Wg2, np.float32)
    bg2 = np.asarray(bg2, np.float32)

    nc = _build_program()
    in_maps = _make_in_maps(pos, Wf1, bf1, Wf2, bf2, Wg1, bg1, Wg2, bg2)

    res = run_bass_kernel_spmd(nc, in_maps, list(range(NCORES)))
    outs = res.results

    affordances = np.empty((N, 16), np.float32)
    reconstructed = np.empty((N, 3), np.float32)
    coherence_spatial = np.empty((N,), np.float32)

    seg_aff = np.zeros((B, 16), np.float64)
    seg_err = np.zeros((B,), np.float64)
    counts = np.bincount(batch, minlength=B).astype(np.float64)
    starts = np.searchsorted(batch, np.arange(B + 1))

    for c in range(NCORES):
        o = outs[c]
        aff0 = _unperm_aff(o["aff"])[:NC]
        diff = _unperm_diff(o["diff"])[:NC]
        err = _unperm_err(o["errp"])[:NC]
        lo = c * NC
        affordances[lo:lo + NC] = aff0 + bf2[None, :]
        reconstructed[lo:lo + NC] = pos[lo:lo + NC] - diff
        coherence_spatial[lo:lo + NC] = err

        parts = o["part"].reshape(NW, 288)
        aff_w = parts[:, 0:256].reshape(NW, 16, 16).sum(axis=1)
        err_w = parts[:, 256:288].sum(axis=1)

        for w in range(NW):
            g0 = lo + w * WINDOW
            g1 = min(g0 + WINDOW, lo + NC)
            s_lo = batch[g0]
            s_hi = batch[g1 - 1]
            full = (g1 - g0) == WINDOW
            if full and s_lo == s_hi:
                seg_aff[s_lo] += aff_w[w].astype(np.float64)
                seg_err[s_lo] += float(err_w[w])
            else:
                for s in range(s_lo, s_hi + 1):
                    a = max(g0, starts[s])
                    b_ = min(g1, starts[s + 1])
                    if b_ > a:
                        seg_aff[s] += aff0[a - lo:b_ - lo].sum(
                            axis=0, dtype=np.float64)
                        seg_err[s] += err[a - lo:b_ - lo].sum(
                            dtype=np.float64)

    denom = np.maximum(counts, 1.0)
    coherence_signal = (seg_err / denom).astype(np.float32)[:, None]
    batch_aff = (seg_aff / denom[:, None]).astype(np.float32) + bf2[None, :]

    # tiny GRU + heads on host (B=64)
    Wx = np.asarray(Wx, np.float32)
    Wh = np.asarray(Wh, np.float32)
    bx = np.asarray(bx, np.float32)
    bh = np.asarray(bh, np.float32)
    Wlat = np.asarray(Wlat, np.float32)
    blat = np.asarray(blat, np.float32)
    Wact = np.asarray(Wact, np.float32)
    bact = np.asarray(bact, np.float32)

    gx = batch_aff @ Wx + bx
    gh = agent_h @ Wh + bh
    AH = agent_h.shape[1]
    gx_r, gx_z, gx_n = gx[:, :AH], gx[:, AH:2 * AH], gx[:, 2 * AH:]
    gh_r, gh_z, gh_n = gh[:, :AH], gh[:, AH:2 * AH], gh[:, 2 * AH:]

    def sigmoid(v):
        return 1.0 / (1.0 + np.exp(-v))

    r = sigmoid(gx_r + gh_r)
    z = sigmoid(gx_z + gh_z)
    n_ = np.tanh(gx_n + r * gh_n)
    agent_h_next = (1.0 - z) * n_ + z * agent_h
    latent = np.tanh(agent_h_next @ Wlat + blat)
    agent_action = latent @ Wact + bact

    return (affordances, reconstructed, coherence_signal.astype(np.float32),
            coherence_spatial, agent_action.astype(np.float32),
            agent_h_next.astype(np.float32))


# revision 19
# speedup vs baseline: 1.9122x; 1.2083x over previous
"""Trainium2 Bass kernel for nn_AdjunctionModel (segment_reduce).

Math (per point, N=1e6 points, B=64 sorted segments):
    h1   = relu(pos @ Wf1 + bf1)            (N,128)
    aff  = h1 @ Wf2 + bf2                   (N,16)   [output]
    h2   = relu(aff @ Wg1 + bg1)            (N,128)
    rec  = h2 @ Wg2 + bg2                   (N,3)    [output]
    err  = sum((pos - rec)^2, -1)           (N,)     [output]
    per-segment means of err and aff feed a tiny GRU (B=64).

Key algebraic fold: there is no nonlinearity between the two middle
matmuls, so  h2 = relu(h1 @ (Wf2 @ Wg1) + (bf2 @ Wg1 + bg1)).  The
device computes, per 512-point block:
    L1   : h1 = Wf1^T @ posT            (fp32r matmul, N=512)
    relu1: s1 = relu(h1 + bf1)          (ACT, fp16 out)
    L3   : pre2 = W23^T @ s1            (fp16 matmul)
    relu2: s2 = relu(pre2 + b23)        (ACT or DVE, alternating)
    aff0 : Wf2p^T @ s1  -> quadrant 32q of a psum tile   (fp16)
    rec0 : Wg2p^T @ s2  -> quadrant 32q of a second tile (fp16)
Per triad (3 blocks), a DVE 32x32 StreamTranspose turns the quadrant-
packed (96,512) psum tiles into per-point-layout and the per-window
(2 triads) partial sums are reduced by a ones-matmul.  Host combines
per-window partials into per-segment sums (recomputing the few windows
that straddle a segment boundary from the per-point outputs), adds the
deferred biases, and runs the tiny GRU in numpy.

Sharding: data-parallel over points, 8 cores, same NEFF on every core
(per-core inputs differ only in data).
"""

import os
import sys
from contextlib import ExitStack

import numpy as np

sys.path.insert(0, "/opt/trn_rl_repo")

import ml_dtypes  # noqa: E402
import concourse.bass as bass  # noqa: E402
import concourse.tile as tile  # noqa: E402
from concourse.tile import add_dep_helper  # noqa: E402
from concourse import bacc, mybir  # noqa: E402
from concourse.bass_utils import run_bass_kernel_spmd  # noqa: E402

F32 = mybir.dt.float32
F32R = mybir.dt.float32r
F16 = mybir.dt.float16
BF16 = mybir.dt.bfloat16
HID_DT = BF16 if os.environ.get("KHID", "f16") == "bf16" else F16
AF = mybir.ActivationFunctionType
ALU = mybir.AluOpType
AX = mybir.AxisListType

N = 1_000_000
B = 64
NCORES = 8
NC = N // NCORES           # 125000 points per core
BLOCK = 512
TRIAD = 3 * BLOCK          # 1536
WINDOW = 2 * TRIAD         # 3072
NW = (NC + WINDOW - 1) // WINDOW   # 41
NPAD = NW * WINDOW         # 125952
NTRIAD = 2 * NW            # 82

# relu2 engine assignment: ACT on even blocks, DVE on odd (≈0.5 split)
def _relu2_on_act(blk: int) -> bool:
    return blk % 2 == 0


_CACHE = {}


def _build_program():
    if "prog" in _CACHE:
        return _CACHE["prog"]

    nc = bacc.Bacc("TRN2", target_bir_lowering=False, debug=False,
                   num_devices=NCORES)

    # ---- DRAM I/O ----
    d_posT = nc.dram_tensor("posT", [3, NPAD], F16, kind="ExternalInput").ap()
    d_posb = nc.dram_tensor("posb", [NW, 96, 96], F32,
                            kind="ExternalInput").ap()
    d_Wf1 = nc.dram_tensor("Wf1", [3, 128], F16, kind="ExternalInput").ap()
    d_W23 = nc.dram_tensor("W23", [128, 128], HID_DT, kind="ExternalInput").ap()
    d_Wf2p = nc.dram_tensor("Wf2p", [128, 32], HID_DT, kind="ExternalInput").ap()
    d_Wg2p = nc.dram_tensor("Wg2p", [128, 32], HID_DT, kind="ExternalInput").ap()
    d_bf1 = nc.dram_tensor("bf1", [128, 1], F32, kind="ExternalInput").ap()
    d_b23 = nc.dram_tensor("b23", [128, 1], F32, kind="ExternalInput").ap()

    d_aff = nc.dram_tensor("aff", [NTRIAD, 96, 16, 16], F32,
                           kind="ExternalOutput").ap()
    d_diff = nc.dram_tensor("diff", [NTRIAD, 96, 48], F32,
                            kind="ExternalOutput").ap()
    d_errp = nc.dram_tensor("errp", [NW, 96, 32], F32,
                            kind="ExternalOutput").ap()
    d_part = nc.dram_tensor("part", [1, NW * 288], F32,
                            kind="ExternalOutput").ap()

    with tile.TileContext(nc) as tc, ExitStack() as ctx:
        consts = ctx.enter_context(tc.tile_pool(name="consts", bufs=1))
        pposT = ctx.enter_context(tc.tile_pool(name="pposT", bufs=6))
        pposb = ctx.enter_context(tc.tile_pool(name="pposb", bufs=6))
        ps1 = ctx.enter_context(tc.tile_pool(name="ps1", bufs=7))
        ps2 = ctx.enter_context(tc.tile_pool(name="ps2", bufs=7))
        pT = ctx.enter_context(tc.tile_pool(name="pT", bufs=2))
        pdiff = ctx.enter_context(tc.tile_pool(name="pdiff", bufs=2))
        pacc = ctx.enter_context(tc.tile_pool(name="pacc", bufs=6))
        ppart = ctx.enter_context(tc.tile_pool(name="ppart", bufs=1))
        psAh = ctx.enter_context(tc.tile_pool(name="psAh", bufs=2,
                                              space="PSUM"))
        psF = ctx.enter_context(tc.tile_pool(name="psF", bufs=1,
                                             space="PSUM"))
        psP = ctx.enter_context(tc.tile_pool(name="psP", bufs=3,
                                             space="PSUM"))
        psQ = ctx.enter_context(tc.tile_pool(name="psQ", bufs=1,
                                             space="PSUM"))
        psR = ctx.enter_context(tc.tile_pool(name="psR", bufs=1,
                                             space="PSUM"))

        Wf1 = consts.tile([3, 128], F16)
        nc.sync.dma_start(out=Wf1, in_=d_Wf1)
        W23 = consts.tile([128, 128], HID_DT)
        nc.sync.dma_start(out=W23, in_=d_W23)
        Wf2p = consts.tile([128, 32], HID_DT)
        nc.sync.dma_start(out=Wf2p, in_=d_Wf2p)
        Wg2p = consts.tile([128, 32], HID_DT)
        nc.sync.dma_start(out=Wg2p, in_=d_Wg2p)
        bf1 = consts.tile([128, 1], F32)
        nc.sync.dma_start(out=bf1, in_=d_bf1)
        b23 = consts.tile([128, 1], F32)
        nc.sync.dma_start(out=b23, in_=d_b23)
        ones = consts.tile([96, 1], F32)
        nc.vector.memset(ones[:], 1.0)

        partials = ppart.tile([1, NW * 288], F32)

        _pe_prev = [None]

        def mm(out, lhsT, rhs):
            r = nc.tensor.matmul(out, lhsT, rhs, start=True, stop=True)
            if _pe_prev[0] is not None and os.environ.get("KCHAIN", "1") == "1":
                add_dep_helper(r.ins, _pe_prev[0],
                               sync=os.environ.get("KSYNC", "0") == "1",
                               reason="pe stream order")
            _pe_prev[0] = r.ins
            return r

        def phase_break():
            if os.environ.get("KPBREAK", "0") == "1":
                _pe_prev[0] = None

        # Phase-batched software pipeline over triad-slots k:
        #   [L1 x3 (k)] [L3' x3 (k-1)] [aff x3 (k-1)] [rec x3 (k-2)]
        # Same stationary weights within each phase -> long dense PE
        # bursts (HAM-warm); every matmul input produced >=1 slot earlier.
        tstate = {}         # per-triad: s1 list, s2 list, aff3, rec3
        win_state = {}      # per-window: posT_w, posb_w, acc

        def load_window(w):
            posT_w = pposT.tile([3, WINDOW], F16, tag="posT")
            nc.sync.dma_start(
                out=posT_w,
                in_=d_posT[:, WINDOW * w:WINDOW * (w + 1)])
            posb_w = pposb.tile([96, 96], F32, tag="posb")
            nc.sync.dma_start(out=posb_w, in_=d_posb[w])
            acc = pacc.tile([96, 288], F32, tag="acc")
            win_state[w] = (posT_w, posb_w, acc)

        def front(k):
            # L1 x3 + relu1 x3 for triad k
            w, t = k // 2, k % 2
            for ww in (w, w + 1):
                if ww < NW and ww not in win_state:
                    load_window(ww)
            posT_w = win_state[w][0]
            phase_break()
            h1s = []
            for q in range(3):
                cs = BLOCK * (3 * t + q)
                h1 = psAh.tile([128, BLOCK], F32, tag="h1")
                mm(h1[:], Wf1[:], posT_w[:, cs:cs + BLOCK])
                h1s.append(h1)
            s1s = []
            for q in range(3):
                s1 = ps1.tile([128, BLOCK], HID_DT, tag="s1")
                nc.scalar.activation(s1[:], h1s[q][:], AF.Relu, bias=bf1[:])
                s1s.append(s1)
            tstate[k] = {"s1": s1s}

        def mid(k):
            # L3' x3 (+relu2) then aff x3, T_aff, aff DMA, acc for triad k
            w, t = k // 2, k % 2
            st = tstate[k]
            phase_break()
            pre2s = []
            for q in range(3):
                pre2 = psP.tile([128, BLOCK], F32, tag="pre2")
                mm(pre2[:], W23[:], st["s1"][q][:])
                pre2s.append(pre2)
            s2s = []
            for q in range(3):
                s2 = ps2.tile([128, BLOCK], HID_DT, tag="s2")
                if _relu2_on_act(3 * k + q):
                    nc.scalar.activation(s2[:], pre2s[q][:], AF.Relu,
                                         bias=b23[:])
                else:
                    nc.vector.tensor_scalar(s2[:], pre2s[q][:], b23[:], 0.0,
                                            ALU.add, ALU.max)
                s2s.append(s2)
            st["s2"] = s2s
            phase_break()
            aff3 = psQ.tile([96, BLOCK], F32, name="aff3", tag="affq")
            for q in range(3):
                mm(aff3[32 * q:32 * q + 32, :], Wf2p[:], st["s1"][q][:])
            st["s1"] = None
            acc = win_state[w][2]
            T_aff = pT.tile([96, BLOCK], F32, tag="Taff")
            nc.vector.transpose(T_aff[:], aff3[:])
            aff_src = T_aff[:, :].rearrange("p (j a) -> p j a",
                                            a=32)[:, :, 0:16]
            nc.sync.dma_start(out=d_aff[k], in_=aff_src)
            if t == 0:
                nc.vector.tensor_copy(acc[:, 0:256], aff_src)
            else:
                nc.vector.tensor_tensor(acc[:, 0:256], acc[:, 0:256],
                                        aff_src, ALU.add)

        def do_flush(w):
            acc = win_state[w][2]
            fl = psF.tile([1, 288], F32, name="flush", tag="flush")
            # not chained: deps are 2 slots old, scheduler places it freely
            nc.tensor.matmul(fl[0:1, :], ones[:], acc[:],
                             start=True, stop=True)
            nc.scalar.activation(partials[0:1, 288 * w:288 * (w + 1)],
                                 fl[0:1, :], AF.Copy)
            nc.sync.dma_start(out=d_errp[w], in_=acc[:, 256:288])
            del win_state[w]

        def recphase(k):
            # rec x3, T_rec, diff, sq, err-reduce for triad k
            w, t = k // 2, k % 2
            if k >= 3 and k % 2 == 1:
                do_flush(k // 2 - 1)
            st = tstate[k]
            phase_break()
            rec3 = psR.tile([96, BLOCK], F32, name="rec3", tag="rec3")
            for q in range(3):
                mm(rec3[32 * q:32 * q + 32, :], Wg2p[:], st["s2"][q][:])
            del tstate[k]
            posb_w, acc = win_state[w][1], win_state[w][2]
            T_rec = pT.tile([96, BLOCK], F32, tag="Trec")
            nc.vector.transpose(T_rec[:], rec3[:])
            rec_src = T_rec[:, :].rearrange("p (j c) -> p j c",
                                            c=32)[:, :, 0:3]
            diff = pdiff.tile([96, 48], F32, tag="diff")
            nc.vector.tensor_tensor(diff[:], posb_w[:, 48 * t:48 * (t + 1)],
                                    rec_src, ALU.subtract)
            nc.sync.dma_start(out=d_diff[k], in_=diff[:])
            sq = pdiff.tile([96, 48], F32, tag="sq")
            nc.scalar.activation(sq[:], diff[:], AF.Square)
            nc.vector.reduce_sum(
                acc[:, 256 + 16 * t:256 + 16 * (t + 1)],
                sq[:, :].rearrange("p (j c) -> p j c", c=3),
                axis=AX.X)

        for k in range(NTRIAD + 2):
            if k < NTRIAD:
                front(k)
            if 1 <= k <= NTRIAD:
                mid(k - 1)
            if 2 <= k <= NTRIAD + 1:
                recphase(k - 2)
        do_flush(NW - 1)

        nc.sync.dma_start(out=d_part, in_=partials[:])

    nc.compile()
    _CACHE["prog"] = nc
    return nc


def _host_prep(pos, bg2):
    """Per-core input arrays from the full pos."""
    in_maps = []
    for c in range(NCORES):
        chunk = np.zeros((NPAD, 3), np.float32)
        chunk[:NC] = pos[c * NC:(c + 1) * NC]
        posT = np.ascontiguousarray(chunk.T.astype(np.float16))
        posb = (chunk - bg2[None, :]).reshape(NTRIAD, 3, 16, 32, 3)
        posb = np.ascontiguousarray(posb.transpose(0, 1, 3, 2, 4)
                                    ).reshape(NTRIAD, 96, 48)
        posb = np.ascontiguousarray(
            posb.reshape(NW, 2, 96, 48).transpose(0, 2, 1, 3)
        ).reshape(NW, 96, 96)
        in_maps.append({"posT": posT, "posb": posb})
    return in_maps


def _unperm_aff(aff_perm):
    # (NTRIAD, 96, 16, 16) [t, (q p), j, a] -> (NPAD, 16)
    a = aff_perm.reshape(NTRIAD, 3, 32, 16, 16).transpose(0, 1, 3, 2, 4)
    return np.ascontiguousarray(a).reshape(NPAD, 16)


def _unperm_diff(diff_perm):
    # (NTRIAD, 96, 48) [t, (q p), (j c)] -> (NPAD, 3)
    d = diff_perm.reshape(NTRIAD, 3, 32, 16, 3).transpose(0, 1, 3, 2, 4)
    return np.ascontiguousarray(d).reshape(NPAD, 3)


def _unperm_err(errp):
    # (NW, 96, 32) [w, (q p), (t j)] -> (NPAD,)
    e = errp.reshape(NW, 3, 32, 2, 16).transpose(0, 3, 1, 4, 2)
    return np.ascontiguousarray(e).reshape(NPAD)




def _make_in_maps(pos, Wf1, bf1, Wf2, bf2, Wg1, bg1, Wg2, bg2):
    # folded middle matmul + deferred biases
    W23 = (Wf2.astype(np.float64) @ Wg1.astype(np.float64)).astype(np.float32)
    b23 = (bf2.astype(np.float64) @ Wg1.astype(np.float64)
           + bg1.astype(np.float64)).astype(np.float32)

    np_hid = ml_dtypes.bfloat16 if os.environ.get("KHID", "f16") == "bf16" \
        else np.float16
    Wf2p = np.zeros((128, 32), np_hid)
    Wf2p[:, 0:16] = Wf2.astype(np_hid)
    Wg2p = np.zeros((128, 32), np_hid)
    Wg2p[:, 0:3] = Wg2.astype(np_hid)

    common = {
        "Wf1": np.ascontiguousarray(Wf1.astype(np.float16)),
        "W23": W23.astype(np_hid),
        "Wf2p": Wf2p,
        "Wg2p": Wg2p,
        "bf1": np.ascontiguousarray(bf1.reshape(128, 1)),
        "b23": np.ascontiguousarray(b23.reshape(128, 1)),
    }
    in_maps = _host_prep(pos, bg2)
    for m in in_maps:
        m.update(common)
    return in_maps




def kernel(pos, batch, agent_h, coherence_signal_prev, coherence_spatial_prev,
           Wf1, bf1, Wf2, bf2, Wg1, bg1, Wg2, bg2,
           Wx, Wh, bx, bh, Wlat, blat, Wact, bact):
    pos = np.asarray(pos, np.float32)
    batch = np.asarray(batch, np.int32)
    agent_h = np.asarray(agent_h, np.float32)
    Wf1 = np.asarray(Wf1, np.float32)
    bf1 = np.asarray(bf1, np.float32)
    Wf2 = np.asarray(Wf2, np.float32)
    bf2 = np.asarray(bf2, np.float32)
    Wg1 = np.asarray(Wg1, np.float32)
    bg1 = np.asarray(bg1, np.float32)
    Wg2 = np.asarray(Wg2, np.float32)
    bg2 = np.asarray(bg2, np.float32)

    nc = _build_program()
    in_maps = _make_in_maps(pos, Wf1, bf1, Wf2, bf2, Wg1, bg1, Wg2, bg2)

    res = run_bass_kernel_spmd(nc, in_maps, list(range(NCORES)))
    outs = res.results

    affordances = np.empty((N, 16), np.float32)
    reconstructed = np.empty((N, 3), np.float32)
    coherence_spatial = np.empty((N,), np.float32)

    seg_aff = np.zeros((B, 16), np.float64)
    seg_err = np.zeros((B,), np.float64)
    counts = np.bincount(batch, minlength=B).astype(np.float64)
    starts = np.searchsorted(batch, np.arange(B + 1))

    for c in range(NCORES):
        o = outs[c]
        aff0 = _unperm_aff(o["aff"])[:NC]
        diff = _unperm_diff(o["diff"])[:NC]
        err = _unperm_err(o["errp"])[:NC]
        lo = c * NC
        affordances[lo:lo + NC] = aff0 + bf2[None, :]
        reconstructed[lo:lo + NC] = pos[lo:lo + NC] - diff
        coherence_spatial[lo:lo + NC] = err

        parts = o["part"].reshape(NW, 288)
        aff_w = parts[:, 0:256].reshape(NW, 16, 16).sum(axis=1)
        err_w = parts[:, 256:288].sum(axis=1)

        for w in range(NW):
            g0 = lo + w * WINDOW
            g1 = min(g0 + WINDOW, lo + NC)
            s_lo = batch[g0]
            s_hi = batch[g1 - 1]
            full = (g1 - g0) == WINDOW
            if full and s_lo == s_hi:
                seg_aff[s_lo] += aff_w[w].astype(np.float64)
                seg_err[s_lo] += float(err_w[w])
            else:
                for s in range(s_lo, s_hi + 1):
                    a = max(g0, starts[s])
                    b_ = min(g1, starts[s + 1])
                    if b_ > a:
                        seg_aff[s] += aff0[a - lo:b_ - lo].sum(
                            axis=0, dtype=np.float64)
                        seg_err[s] += err[a - lo:b_ - lo].sum(
                            dtype=np.float64)

    denom = np.maximum(counts, 1.0)
    coherence_signal = (seg_err / denom).astype(np.float32)[:, None]
    batch_aff = (seg_aff / denom[:, None]).astype(np.float32) + bf2[None, :]

    # tiny GRU + heads on host (B=64)
    Wx = np.asarray(Wx, np.float32)
    Wh = np.asarray(Wh, np.float32)
    bx = np.asarray(bx, np.float32)
    bh = np.asarray(bh, np.float32)
    Wlat = np.asarray(Wlat, np.float32)
    blat = np.asarray(blat, np.float32)
    Wact = np.asarray(Wact, np.float32)
    bact = np.asarray(bact, np.float32)

    gx = batch_aff @ Wx + bx
    gh = agent_h @ Wh + bh
    AH = agent_h.shape[1]
    gx_r, gx_z, gx_n = gx[:, :AH], gx[:, AH:2 * AH], gx[:, 2 * AH:]
    gh_r, gh_z, gh_n = gh[:, :AH], gh[:, AH:2 * AH], gh[:, 2 * AH:]

    def sigmoid(v):
        return 1.0 / (1.0 + np.exp(-v))

    r = sigmoid(gx_r + gh_r)
    z = sigmoid(gx_z + gh_z)
    n_ = np.tanh(gx_n + r * gh_n)
    agent_h_next = (1.0 - z) * n_ + z * agent_h
    latent = np.tanh(agent_h_next @ Wlat + blat)
    agent_action = latent @ Wact + bact

    return (affordances, reconstructed, coherence_signal.astype(np.float32),
            coherence_spatial, agent_action.astype(np.float32),
            agent_h_next.astype(np.float32))
